# revision 9
# baseline (speedup 1.0000x reference)
"""Gated Linear Attention forward on 8 Trainium2 NeuronCores (Bass/Tile).

Problem: B=4, T=1024, D=1024, H=8, DK=64, DV=128, conv4 on q/k/v, low-rank
log-sigmoid forget gate, recurrent scan, RMS-norm + swish output gate, out proj.

Sharding: core = 2*b + hg  (b = batch, hg = half of the heads).
Each core computes its batch's tokens for 4 heads end-to-end and a partial
output projection (Wo row-block); the host sums the two partials per batch.

v2 (bf16): all matmul operands and elementwise intermediates are bf16
(fp32 PSUM accumulation).  Relative to the f32r baseline this
 - halves PE streaming time (f32r "HIGH" mode measured ~2 cycles/row),
 - enables fast weight load, halves transpose cost,
 - doubles DVE throughput on SBUF-only elementwise ops,
 - halves DMA bytes.
All weights arrive host-packed in [128, ...] partition-major layouts so every
DMA moves >=2KB contiguous per partition (the old rearranged/convdiag DMAs
moved 512B packets and crawled).  Conv diag matrices are built on-device from
the identity (saves a 2MB host DMA).  silu runs as a single ACT op, and the
ACT schedule needs only 3 table loads (nle -> silu -> nle).  The RMS-norm,
output gate, transpose and output projection are fused per chunk so the
output streams to DRAM throughout the chunk loop.
"""

import numpy as np
import ml_dtypes

import concourse.bass as bass
import concourse.mybir as mybir
import concourse.tile as tile
from concourse import bacc
from concourse.bass_utils import run_bass_kernel_spmd

F32 = mybir.dt.float32
BF = mybir.dt.bfloat16
AF = mybir.ActivationFunctionType
OP = mybir.AluOpType

# problem constants (hardcoded per the task contract)
B, T, D, H = 4, 1024, 1024, 8
KD, VD = 512, 1024
DK, DV = 64, 128
CONV = 4
GATE_NORM = 16.0
EPS = 1e-5
LN8 = float(np.log(8.0))

# per-core shapes
KDC, VDC = 256, 512          # q/k and v/gate channels per core
MIQ, MIV = 2, 4              # 128-wide channel tiles for q/k and v
C, NCH = 128, 8              # chunk length, number of chunks
G = 2                        # head groups of 2 heads (128 chans) per core
NCORES = 8


def build_program():
    nc = bacc.Bacc("TRN2", target_bir_lowering=False, debug=False)

    # ---- DRAM I/O (all host-packed, partition-major, contiguous rows) ------
    srcT_d = nc.dram_tensor("srcT_in", [128, 8 * T], BF, kind="ExternalInput")
    wq_d = nc.dram_tensor("wq", [128, 8 * 256], BF, kind="ExternalInput")
    wk_d = nc.dram_tensor("wk", [128, 8 * 256], BF, kind="ExternalInput")
    wv_d = nc.dram_tensor("wv", [128, 8 * 512], BF, kind="ExternalInput")
    wgate_d = nc.dram_tensor("wgate", [128, 8 * 512], BF, kind="ExternalInput")
    wg1_d = nc.dram_tensor("wg1", [128, 8 * 16], BF, kind="ExternalInput")
    wg2b_d = nc.dram_tensor("wg2b", [17, KDC], BF, kind="ExternalInput")
    wo_d = nc.dram_tensor("wo", [128, 4 * 1024], BF, kind="ExternalInput")
    convw_d = nc.dram_tensor("convw", [128, 32], F32, kind="ExternalInput")
    maskc_d = nc.dram_tensor("maskc", [128, NCH], F32, kind="ExternalInput")
    out_d = nc.dram_tensor("out", [T, D], F32, kind="ExternalOutput")

    ident_np = np.eye(128, dtype=ml_dtypes.bfloat16)
    u = np.triu(np.ones((128, 128), np.float32)).astype(ml_dtypes.bfloat16)
    ident_d = nc.inline_tensor(ident_np, "ident_c")
    triu2_d = nc.inline_tensor(np.concatenate([u, u], axis=1), "triu2_c")

    # ---- static SBUF -------------------------------------------------------
    srcT = nc.alloc_sbuf_tensor("srcT", [128, 8, T], BF)         # src^T, d-major
    q_sb = nc.alloc_sbuf_tensor("q_sb", [128, MIQ, T], BF)       # q then q~
    k_sb = nc.alloc_sbuf_tensor("k_sb", [128, MIQ, T], BF)       # k then k~
    v_sb = nc.alloc_sbuf_tensor("v_sb", [128, MIV, T], BF)
    gate_sb = nc.alloc_sbuf_tensor("gate_sb", [128, NCH, VDC], BF)
    xgT = nc.alloc_sbuf_tensor("xgT", [17, T], BF)               # (src@Wg1)^T + ones
    spT = nc.alloc_sbuf_tensor("spT", [128, MIQ, T], F32)        # softplus(-logit)
    bsum = nc.alloc_sbuf_tensor("bsum", [128, MIQ, T], F32)      # per-chunk cumsum
    texpq = nc.alloc_sbuf_tensor("texpq", [128, MIQ, T], BF)     # exp(-bsum/16)/8
    texpk = nc.alloc_sbuf_tensor("texpk", [128, MIQ, T], BF)     # exp(+bsum/16)
    Eall = nc.alloc_sbuf_tensor("Eall", [128, MIQ, NCH], F32)    # exp(b_C) per chunk
    ssq_all = nc.alloc_sbuf_tensor("ssq_all", [128, NCH * 4], F32)
    lnr_all = nc.alloc_sbuf_tensor("lnr_all", [128, NCH * 4], F32)
    rr_all = nc.alloc_sbuf_tensor("rr_all", [128, NCH * 4], F32)
    wq_sb = nc.alloc_sbuf_tensor("wq_sb", [128, 8, 256], BF)
    wk_sb = nc.alloc_sbuf_tensor("wk_sb", [128, 8, 256], BF)
    wv_sb = nc.alloc_sbuf_tensor("wv_sb", [128, 8, 512], BF)
    wgate_sb = nc.alloc_sbuf_tensor("wgate_sb", [128, 8, 512], BF)
    wg1_sb = nc.alloc_sbuf_tensor("wg1_sb", [128, 8, 16], BF)
    wg2b_sb = nc.alloc_sbuf_tensor("wg2b_sb", [17, KDC], BF)
    wo_sb = nc.alloc_sbuf_tensor("wo_sb", [128, 4, 1024], BF)
    convw_sb = nc.alloc_sbuf_tensor("convw_sb", [128, 32], F32)
    dg_sb = nc.alloc_sbuf_tensor("dg_sb", [128, 32, 128], BF)    # diag(conv w)
    maskc_sb = nc.alloc_sbuf_tensor("maskc_sb", [128, NCH], F32)
    ident = nc.alloc_sbuf_tensor("ident", [128, 128], BF)
    triu2 = nc.alloc_sbuf_tensor("triu2", [128, 256], BF)
    ones_sb = nc.alloc_sbuf_tensor("ones_sb", [128, 128], F32)
    Sblk = [nc.alloc_sbuf_tensor(f"Sblk{g}", [128, 256], BF) for g in range(G)]
    qblk = [nc.alloc_sbuf_tensor(f"qblk{g}", [128, 256], BF) for g in range(G)]
    negln8 = nc.alloc_sbuf_tensor("negln8", [128, 1], F32)
    eps_col = nc.alloc_sbuf_tensor("eps_col", [128, 1], F32)

    with tile.TileContext(nc) as tc:
        with (
            tc.tile_pool(name="scr", bufs=4) as scr,
            tc.tile_pool(name="scr2", bufs=4) as scr2,
            tc.tile_pool(name="stage", bufs=3) as stage_pool,
            tc.tile_pool(name="ps_big", bufs=2, space="PSUM") as ps_big,
            tc.tile_pool(name="ps_sm", bufs=4, space="PSUM") as ps_sm,
            tc.tile_pool(name="ps_op", bufs=2, space="PSUM") as ps_op,
        ):
            # ---- phase 0: constants + small weights (sync queue) -----------
            nc.sync.dma_start(out=ident[:], in_=ident_d[:])
            nc.sync.dma_start(out=triu2[:], in_=triu2_d[:])
            nc.sync.dma_start(out=wg1_sb[:], in_=wg1_d[:].rearrange("p (kt m) -> p kt m", m=16))
            nc.sync.dma_start(out=wg2b_sb[:], in_=wg2b_d[:])
            nc.sync.dma_start(out=convw_sb[:], in_=convw_d[:])
            nc.sync.dma_start(out=maskc_sb[:], in_=maskc_d[:])
            nc.vector.memset(ones_sb[:], 1.0)
            nc.vector.memset(xgT[:], 1.0)   # row 16 = bias row; 0..15 overwritten
            nc.vector.memset(negln8[:], -LN8)
            nc.vector.memset(eps_col[:], EPS)
            for g in range(G):
                nc.vector.memset(Sblk[g][:], 0.0)
                nc.vector.memset(qblk[g][:], 0.0)

            # ---- src: 8 tile DMAs, round-robin over the DMA engines --------
            dma_engs = [nc.sync, nc.scalar, nc.gpsimd]
            for kt in range(8):
                dma_engs[kt % 3].dma_start(
                    out=srcT[:, kt, :], in_=srcT_d[:, kt * T:(kt + 1) * T]
                )
            # q/k weights next (needed second), big late weights last
            nc.scalar.dma_start(out=wq_sb[:], in_=wq_d[:].rearrange("p (kt m) -> p kt m", m=256))
            nc.sync.dma_start(out=wk_sb[:], in_=wk_d[:].rearrange("p (kt m) -> p kt m", m=256))
            nc.gpsimd.dma_start(out=wv_sb[:], in_=wv_d[:].rearrange("p (kt m) -> p kt m", m=512))
            nc.gpsimd.dma_start(out=wgate_sb[:], in_=wgate_d[:].rearrange("p (kt m) -> p kt m", m=512))
            nc.gpsimd.dma_start(out=wo_sb[:], in_=wo_d[:].rearrange("p (h m) -> p h m", m=1024))

            # ---- gk path: xg = (src@Wg1)^T, logits, softplus ---------------
            for nh in range(2):
                p = ps_big.tile([16, 512], F32, name="pp_xg", tag="ppb")
                for kt in range(8):
                    nc.tensor.matmul(
                        p[:], wg1_sb[:, kt, :], srcT[:, kt, nh * 512:(nh + 1) * 512],
                        start=(kt == 0), stop=(kt == 7),
                    )
                nc.vector.tensor_copy(out=xgT[0:16, nh * 512:(nh + 1) * 512], in_=p[:])
            # spT = softplus(-logit) = ln(1 + exp(-logit)); all Exps before
            # all Lns (same nle table set, better ACT pipelining)
            enxs = []
            for mi in range(MIQ):
                for nh in range(2):
                    p = ps_big.tile([128, 512], F32, name="pp_sp", tag="ppb")
                    nc.tensor.matmul(
                        p[:], wg2b_sb[:, mi * 128:(mi + 1) * 128],
                        xgT[:, nh * 512:(nh + 1) * 512], start=True, stop=True,
                    )
                    enx = scr2.tile([128, 512], BF, name="enx", tag="enx", bufs=4)
                    nc.scalar.activation(enx[:], p[:], AF.Exp, scale=-1.0)
                    enxs.append((mi, nh, enx))
            for mi, nh, enx in enxs:
                nc.scalar.activation(
                    spT[:, mi, nh * 512:(nh + 1) * 512], enx[:], AF.Ln, bias=1.0,
                )
            # per-chunk inclusive cumsum; chunk-end decay factors
            for mi in range(MIQ):
                for c in range(NCH):
                    csl = slice(c * 128, (c + 1) * 128)
                    nc.vector.tensor_tensor_scan(
                        out=bsum[:, mi, csl], data0=ones_sb[:], data1=spT[:, mi, csl],
                        initial=0.0, op0=OP.mult, op1=OP.add,
                    )
                ends = bsum[:, mi, :].rearrange("p (c s) -> p c s", s=128)[:, :, 127:128]
                bce = scr.tile([128, 8], F32, name="bce", tag="bce")
                nc.vector.tensor_copy(
                    out=bce[:].rearrange("p (c one) -> p c one", one=1), in_=ends
                )
                nc.scalar.activation(
                    Eall[:, mi, :], bce[:], AF.Exp, scale=-1.0 / GATE_NORM,
                )
                # q~ factor exp(b)/8 and k~ factor exp(-b), full-T
                nc.scalar.activation(
                    texpq[:, mi, :], bsum[:, mi, :], AF.Exp,
                    scale=-1.0 / GATE_NORM, bias=negln8[:],
                )
                nc.scalar.activation(
                    texpk[:, mi, :], bsum[:, mi, :], AF.Exp, scale=1.0 / GATE_NORM,
                )

            # conv diag matrices from identity (per channel-tile, per tap)
            for t4 in range(32):
                nc.vector.tensor_scalar_mul(
                    dg_sb[:, t4, :], ident[:], convw_sb[:, t4:t4 + 1]
                )

            def conv_proj(w_sb, dst, mi_count, ti_base):
                """dst[:, mi, :] = silu(conv4(src @ W[:, mi-block]))^T."""
                for mi in range(mi_count):
                    pre = scr2.tile([128, 1028], BF, name="pre", tag="pre")
                    nc.gpsimd.memset(pre[:, 0:4], 0.0)
                    for nh in range(2):
                        p = ps_big.tile([128, 512], F32, name="pp_c", tag="ppb")
                        for kt in range(8):
                            nc.tensor.matmul(
                                p[:], w_sb[:, kt, mi * 128:(mi + 1) * 128],
                                srcT[:, kt, nh * 512:(nh + 1) * 512],
                                start=(kt == 0), stop=(kt == 7),
                            )
                        nc.vector.tensor_copy(
                            out=pre[:, 4 + nh * 512:4 + (nh + 1) * 512], in_=p[:]
                        )
                    for nh in range(2):
                        cp = ps_big.tile([128, 512], F32, name="cp", tag="ppb")
                        for j in range(CONV):
                            nc.tensor.matmul(
                                cp[:], dg_sb[:, (ti_base + mi) * 4 + j, :],
                                pre[:, 1 + nh * 512 + j:1 + nh * 512 + j + 512],
                                start=(j == 0), stop=(j == 3),
                            )
                        sg = scr2.tile([128, 512], BF, name="sg", tag="sg")
                        nc.scalar.activation(sg[:], cp[:], AF.Sigmoid)
                        nc.vector.tensor_mul(
                            dst[:, mi, nh * 512:(nh + 1) * 512], cp[:], sg[:]
                        )

            conv_proj(wq_sb, q_sb, MIQ, 0)
            conv_proj(wk_sb, k_sb, MIQ, MIQ)
            # q~ / k~ in place (bf16 2x-mode DVE)
            for mi in range(MIQ):
                nc.vector.tensor_mul(q_sb[:, mi, :], q_sb[:, mi, :], texpq[:, mi, :])
                nc.vector.tensor_mul(k_sb[:, mi, :], k_sb[:, mi, :], texpk[:, mi, :])
            conv_proj(wv_sb, v_sb, MIV, 2 * MIQ)
            # output gate: silu(src @ Wgate), t-major
            for mt in range(8):
                p = ps_big.tile([128, 512], F32, name="pp_g", tag="ppb")
                for kt in range(8):
                    nc.tensor.matmul(
                        p[:], srcT[:, kt, mt * 128:(mt + 1) * 128], wgate_sb[:, kt, :],
                        start=(kt == 0), stop=(kt == 7),
                    )
                sgg = scr2.tile([128, 512], BF, name="sgg", tag="sg")
                nc.scalar.activation(sgg[:], p[:], AF.Sigmoid)
                nc.vector.tensor_mul(gate_sb[:, mt, :], p[:], sgg[:])

            # ---- GLA chunk recurrence with fused rms/gate/outproj tail -----
            for c in range(NCH):
                csl = slice(c * 128, (c + 1) * 128)
                front = []  # per g: (a_sb, vnat, khnat, e_col)
                for g in range(G):
                    e_col = Eall[:, g, c:c + 1]
                    # k^ = k~ * exp(b_C) (per-partition scalar), pre-transpose
                    kh_s = scr.tile([128, 128], BF, name="kh_s", tag="kh_s")
                    nc.vector.tensor_scalar_mul(kh_s[:], k_sb[:, g, csl], e_col)
                    # A~ for both heads via block-diagonal q (the direct
                    # 64-partition matmul formulation faults on HW)
                    nc.vector.tensor_copy(out=qblk[g][0:64, 0:128], in_=q_sb[0:64, g, csl])
                    nc.vector.tensor_copy(out=qblk[g][64:128, 128:256], in_=q_sb[64:128, g, csl])
                    ps_a = ps_sm.tile([128, 256], F32, name="ps_a", tag="ps_sm")
                    nc.tensor.matmul(
                        ps_a[:], k_sb[:, g, csl], qblk[g][:], start=True, stop=True,
                    )
                    a_sb = scr.tile([128, 256], BF, name="a_sb", tag="a_sb")
                    nc.vector.tensor_mul(a_sb[:], ps_a[:], triu2[:])
                    # v chunk -> time-major (+ padding mask)
                    ps_v = ps_sm.tile([128, 256], BF, name="ps_v", tag="ps_sm")
                    nc.tensor.matmul(
                        ps_v[:, 0:128], v_sb[:, 2 * g, csl], ident[:],
                        is_transpose=True, start=True, stop=False,
                        skip_group_check=True,
                    )
                    nc.tensor.matmul(
                        ps_v[:, 128:256], v_sb[:, 2 * g + 1, csl], ident[:],
                        is_transpose=True, start=False, stop=True,
                        skip_group_check=True,
                    )
                    vnat = scr.tile([128, 256], BF, name="vnat", tag="vnat")
                    nc.vector.tensor_scalar_mul(vnat[:], ps_v[:], maskc_sb[:, c:c + 1])
                    # k^ chunk -> time-major
                    ps_k = ps_sm.tile([128, 128], BF, name="ps_k", tag="ps_sm")
                    nc.tensor.transpose(ps_k[:], kh_s[:], ident[:])
                    khnat = scr.tile([128, 128], BF, name="khnat", tag="khnat")
                    nc.vector.tensor_copy(out=khnat[:], in_=ps_k[:])
                    front.append((a_sb, vnat, khnat, e_col))
                for g in range(G):
                    a_sb, vnat, khnat, e_col = front[g]
                    # o = A~^T v (intra) + q~ @ S (inter)
                    ps_o = ps_op.tile([128, 256], F32, name="ps_o", tag="ps_o")
                    nc.tensor.matmul(
                        ps_o[:, 0:128], a_sb[:, 0:128], vnat[:, 0:128],
                        start=True, stop=False, skip_group_check=True,
                    )
                    nc.tensor.matmul(
                        ps_o[:, 128:256], a_sb[:, 128:256], vnat[:, 128:256],
                        start=False, stop=False, skip_group_check=True,
                    )
                    nc.tensor.matmul(
                        ps_o[:], q_sb[:, g, csl], Sblk[g][:],
                        start=False, stop=True, skip_group_check=True,
                    )
                    # state update: S = diag(E) S + k^T v
                    ps_s = ps_sm.tile([128, 256], F32, name="ps_s", tag="ps_sm")
                    nc.tensor.matmul(ps_s[:], khnat[:], vnat[:], start=True, stop=True)
                    nc.vector.scalar_tensor_tensor(
                        out=Sblk[g][0:64, 0:128], in0=Sblk[g][0:64, 0:128],
                        scalar=e_col[0:64, :], in1=ps_s[0:64, 0:128],
                        op0=OP.mult, op1=OP.add,
                    )
                    nc.vector.scalar_tensor_tensor(
                        out=Sblk[g][64:128, 128:256], in0=Sblk[g][64:128, 128:256],
                        scalar=e_col[64:128, :], in1=ps_s[64:128, 128:256],
                        op0=OP.mult, op1=OP.add,
                    )
                    # per-head sum of squares -> rrms for these 2 heads
                    idx = c * 4 + 2 * g
                    for lh in range(2):
                        sqd = scr.tile([128, 128], BF, name="sqd", tag="sqd")
                        nc.scalar.activation(
                            sqd[:], ps_o[:, lh * 128:(lh + 1) * 128], AF.Square,
                            accum_out=ssq_all[:, idx + lh:idx + lh + 1],
                        )
                    nc.scalar.activation(
                        lnr_all[:, idx:idx + 2], ssq_all[:, idx:idx + 2],
                        AF.Ln, scale=1.0 / DV, bias=eps_col[:],
                    )
                    nc.scalar.activation(
                        rr_all[:, idx:idx + 2], lnr_all[:, idx:idx + 2],
                        AF.Exp, scale=-0.5,
                    )
                    # gated output: gate_sb <- (o * rrms) * gate  (one stt/head)
                    for lh in range(2):
                        gdst = gate_sb[:, c, g * 256 + lh * 128:g * 256 + (lh + 1) * 128]
                        nc.vector.scalar_tensor_tensor(
                            out=gdst, in0=ps_o[:, lh * 128:(lh + 1) * 128],
                            scalar=rr_all[:, idx + lh:idx + lh + 1], in1=gdst,
                            op0=OP.mult, op1=OP.mult,
                        )
                # tail for chunk c: transpose gated o, project, stream out
                ogs = []
                for hp in range(0, 4, 2):
                    ps_g = ps_sm.tile([128, 256], BF, name="ps_g", tag="ps_sm")
                    nc.tensor.matmul(
                        ps_g[:, 0:128], gate_sb[:, c, hp * 128:(hp + 1) * 128],
                        ident[:], is_transpose=True, start=True, stop=False,
                        skip_group_check=True,
                    )
                    nc.tensor.matmul(
                        ps_g[:, 128:256], gate_sb[:, c, (hp + 1) * 128:(hp + 2) * 128],
                        ident[:], is_transpose=True, start=False, stop=True,
                        skip_group_check=True,
                    )
                    og = scr.tile([128, 256], BF, name="og", tag="og")
                    nc.vector.tensor_copy(out=og[:], in_=ps_g[:])
                    ogs.append(og)
                for nh in range(2):
                    p = ps_big.tile([128, 512], F32, name="p_out", tag="ppb")
                    for h in range(4):
                        nc.tensor.matmul(
                            p[:], ogs[h // 2][:, (h % 2) * 128:(h % 2 + 1) * 128],
                            wo_sb[:, h, nh * 512:(nh + 1) * 512],
                            start=(h == 0), stop=(h == 3),
                        )
                    stg = stage_pool.tile([128, 512], F32, name="stage", tag="stage")
                    if nh == 0:
                        nc.vector.tensor_copy(out=stg[:], in_=p[:])
                    else:
                        nc.scalar.copy(out=stg[:], in_=p[:])
                    nc.sync.dma_start(
                        out=out_d[c * 128:(c + 1) * 128, nh * 512:(nh + 1) * 512],
                        in_=stg[:],
                    )

    nc.compile()
    return nc


_NC_CACHE = None


def _get_program():
    global _NC_CACHE
    if _NC_CACHE is None:
        _NC_CACHE = build_program()
    return _NC_CACHE


def shard_inputs(
    src, valid_mask, Wq, Wk, Wv, conv_q_w, conv_k_w, conv_v_w,
    Wg1, Wg2, bg2, Wgate, rms_w, Wo,
):
    """Build the 8 per-core input maps (bf16, partition-major packed)."""
    bf = ml_dtypes.bfloat16
    f = np.float32

    def pack_km(w, m):
        # [1024, m] -> [128, 8*m] with (p, kt*m + j) = w[kt*128 + p, j]
        return np.ascontiguousarray(
            np.asarray(w, f).reshape(8, 128, m).transpose(1, 0, 2).reshape(128, 8 * m)
        ).astype(bf)

    src = np.asarray(src, f)
    valid_mask = np.asarray(valid_mask)
    wo_scaled = np.asarray(Wo, f) * np.tile(np.asarray(rms_w, f), VD // DV)[:, None]
    in_maps = []
    for core in range(NCORES):
        b, hg = core // 2, core % 2
        qs = slice(hg * KDC, (hg + 1) * KDC)
        vs = slice(hg * VDC, (hg + 1) * VDC)
        wg2b = np.concatenate(
            [np.asarray(Wg2, f)[:, qs], np.asarray(bg2, f)[None, qs]], axis=0
        )
        # conv taps: [128, 32] with (p, ti*4 + j) = w_ti[p, j]
        convw = np.zeros((128, 32), f)
        ti = 0
        for w, sel, n in ((conv_q_w, qs, MIQ), (conv_k_w, qs, MIQ),
                          (conv_v_w, vs, MIV)):
            wa = np.asarray(w, f)[sel]
            for i in range(n):
                convw[:, ti * 4:(ti + 1) * 4] = wa[i * 128:(i + 1) * 128]
                ti += 1
        # wo rows for this core's vch block: [512, 1024] -> [128, 4*1024]
        wo_core = np.ascontiguousarray(
            wo_scaled[vs].reshape(4, 128, 1024).transpose(1, 0, 2).reshape(128, 4096)
        ).astype(bf)

        in_maps.append({
            "srcT_in": np.ascontiguousarray(
                src[b].T.reshape(8, 128, T).transpose(1, 0, 2).reshape(128, 8 * T)
            ).astype(bf),
            "wq": pack_km(np.asarray(Wq, f)[:, qs], 256),
            "wk": pack_km(np.asarray(Wk, f)[:, qs], 256),
            "wv": pack_km(np.asarray(Wv, f)[:, vs], 512),
            "wgate": pack_km(np.asarray(Wgate, f)[:, vs], 512),
            "wg1": pack_km(np.asarray(Wg1, f), 16),
            "wg2b": np.ascontiguousarray(wg2b).astype(bf),
            "wo": wo_core,
            "convw": convw,
            "maskc": np.ascontiguousarray(
                valid_mask[b].astype(f).reshape(NCH, 128).T
            ),
        })
    return in_maps


def kernel(**inputs):
    nc = _get_program()
    in_maps = shard_inputs(**inputs)
    res = run_bass_kernel_spmd(nc, in_maps, list(range(NCORES)))
    out = np.zeros((B, T, D), np.float32)
    for core in range(NCORES):
        out[core // 2] += res.results[core]["out"]
    return out


if __name__ == "__main__":
    prog = _get_program()
    print("program built OK")


# revision 10
# speedup vs baseline: 1.2030x; 1.2030x over previous
"""Gated Linear Attention forward on 8 Trainium2 NeuronCores (Bass/Tile).

Problem: B=4, T=1024, D=1024, H=8, DK=64, DV=128, conv4 on q/k/v, low-rank
log-sigmoid forget gate, recurrent scan, RMS-norm + swish output gate, out proj.

Sharding: core = 2*b + hg  (b = batch, hg = half of the heads).
Each core computes its batch's tokens for 4 heads end-to-end and a partial
output projection (Wo row-block); the host sums the two partials per batch.

v4: bf16 matmuls (fp32 PSUM), and:
 - src arrives as 16 half-tiles issued first across all three DMA queues;
   weight DMAs are single straight descriptors (no AP splitting).
 - depthwise conv4 runs on DVE/GpSimd as shifted scalar_tensor_tensor chains
   in fp16 (frees ~20us of PE diag-matmul time).
 - silu is a single ACT op where available; the ACT table schedule is
   exp -> ln -> exp -> silu with NOTHING in the chunk loop but filler
   functions (square/copy), so zero in-loop table loads.
 - the per-chunk RMS rsqrt is computed on DVE (bit-trick seed + 2 Newton
   steps) instead of ACT Ln/Exp, which would force 4 table loads per chunk.
 - the output gate projection is interleaved chunk-by-chunk with the GLA
   recurrence, and each chunk's rms/gate/transpose/out-projection tail is
   emitted one chunk behind the recurrence so the Newton-rr latency stays
   off the critical path; output streams to DRAM throughout.
"""

import numpy as np
import ml_dtypes

import concourse.bass as bass
import concourse.mybir as mybir
import concourse.tile as tile
from concourse import bacc
from concourse.bass_utils import run_bass_kernel_spmd

F32 = mybir.dt.float32
BF = mybir.dt.bfloat16
F16 = mybir.dt.float16
I32 = mybir.dt.int32
AF = mybir.ActivationFunctionType
OP = mybir.AluOpType

B, T, D, H = 4, 1024, 1024, 8
KD, VD = 512, 1024
DK, DV = 64, 128
CONV = 4
GATE_NORM = 16.0
EPS = 1e-5
LN8 = float(np.log(8.0))

KDC, VDC = 256, 512
MIQ, MIV = 2, 4
C, NCH = 128, 8
G = 2
NCORES = 8

RSQRT_MAGIC = 0x5F3759DF
import os as _os
USE_SILU = _os.environ.get("K_NO_SILU") != "1"   # HW-verified; CoreSim lacks Silu


def build_program():
    nc = bacc.Bacc("TRN2", target_bir_lowering=False, debug=False)

    srcT_d = nc.dram_tensor("srcT_in", [128, 8 * T], BF, kind="ExternalInput")
    wq_d = nc.dram_tensor("wq", [128, 8 * 256], BF, kind="ExternalInput")
    wk_d = nc.dram_tensor("wk", [128, 8 * 256], BF, kind="ExternalInput")
    wv_d = nc.dram_tensor("wv", [128, 8 * 512], BF, kind="ExternalInput")
    wgate_d = nc.dram_tensor("wgate", [128, 8 * 512], BF, kind="ExternalInput")
    wg1_d = nc.dram_tensor("wg1", [128, 8 * 16], BF, kind="ExternalInput")
    wg2b_d = nc.dram_tensor("wg2b", [17, KDC], BF, kind="ExternalInput")
    wo_d = nc.dram_tensor("wo", [128, 4 * 1024], BF, kind="ExternalInput")
    convw_d = nc.dram_tensor("convw", [128, 32], F32, kind="ExternalInput")
    maskc_d = nc.dram_tensor("maskc", [128, NCH], F32, kind="ExternalInput")
    out_d = nc.dram_tensor("out", [T, D], F32, kind="ExternalOutput")

    ident_np = np.eye(128, dtype=ml_dtypes.bfloat16)
    u = np.triu(np.ones((128, 128), np.float32)).astype(ml_dtypes.bfloat16)
    ident_d = nc.inline_tensor(ident_np, "ident_c")
    triu2_d = nc.inline_tensor(np.concatenate([u, u], axis=1), "triu2_c")

    srcT = nc.alloc_sbuf_tensor("srcT", [128, 8, T], BF)
    q_sb = nc.alloc_sbuf_tensor("q_sb", [128, MIQ, T], BF)
    k_sb = nc.alloc_sbuf_tensor("k_sb", [128, MIQ, T], BF)
    v_sb = nc.alloc_sbuf_tensor("v_sb", [128, MIV, T], BF)
    gate_sb = nc.alloc_sbuf_tensor("gate_sb", [128, NCH, VDC], BF)
    xgT = nc.alloc_sbuf_tensor("xgT", [17, T], BF)
    spT = nc.alloc_sbuf_tensor("spT", [128, MIQ, T], F32)
    bsum = nc.alloc_sbuf_tensor("bsum", [128, MIQ, T], F32)
    texpq = nc.alloc_sbuf_tensor("texpq", [128, MIQ, T], BF)
    texpk = nc.alloc_sbuf_tensor("texpk", [128, MIQ, T], BF)
    Eall = nc.alloc_sbuf_tensor("Eall", [128, MIQ, NCH], F32)
    ssq_all = nc.alloc_sbuf_tensor("ssq_all", [128, NCH * 4], F32)
    rr_all = nc.alloc_sbuf_tensor("rr_all", [128, NCH * 4], F32)
    wq_sb = nc.alloc_sbuf_tensor("wq_sb", [128, 8, 256], BF)
    wk_sb = nc.alloc_sbuf_tensor("wk_sb", [128, 8, 256], BF)
    wv_sb = nc.alloc_sbuf_tensor("wv_sb", [128, 8, 512], BF)
    wgate_sb = nc.alloc_sbuf_tensor("wgate_sb", [128, 8, 512], BF)
    wg1_sb = nc.alloc_sbuf_tensor("wg1_sb", [128, 8, 16], BF)
    wg2b_sb = nc.alloc_sbuf_tensor("wg2b_sb", [17, KDC], BF)
    wo_sb = nc.alloc_sbuf_tensor("wo_sb", [128, 4, 1024], BF)
    convw_sb = nc.alloc_sbuf_tensor("convw_sb", [128, 32], F32)
    maskc_sb = nc.alloc_sbuf_tensor("maskc_sb", [128, NCH], F32)
    ident = nc.alloc_sbuf_tensor("ident", [128, 128], BF)
    triu2 = nc.alloc_sbuf_tensor("triu2", [128, 256], BF)
    ones_sb = nc.alloc_sbuf_tensor("ones_sb", [128, 128], F32)
    Sblk = [nc.alloc_sbuf_tensor(f"Sblk{g}", [128, 256], BF) for g in range(G)]
    qblk = [nc.alloc_sbuf_tensor(f"qblk{g}", [128, 256], BF) for g in range(G)]
    negln8 = nc.alloc_sbuf_tensor("negln8", [128, 1], F32)
    magic4 = nc.alloc_sbuf_tensor("magic4", [128, 4], I32)
    eps_col = nc.alloc_sbuf_tensor("eps_col", [128, 1], F32)

    with tile.TileContext(nc) as tc:
        with (
            tc.tile_pool(name="scr", bufs=4) as scr,
            tc.tile_pool(name="scr2", bufs=4) as scr2,
            tc.tile_pool(name="nwt", bufs=2) as nwt,
            tc.tile_pool(name="stage", bufs=3) as stage_pool,
            tc.tile_pool(name="ps_big", bufs=2, space="PSUM") as ps_big,
            tc.tile_pool(name="ps_sm", bufs=4, space="PSUM") as ps_sm,
            tc.tile_pool(name="ps_op", bufs=2, space="PSUM") as ps_op,
        ):
            # ---- DMA schedule: src halves first, round-robin over queues ---
            nc.sync.dma_start(out=wg1_sb[:].rearrange("p a b -> p (a b)"), in_=wg1_d[:])
            nc.sync.dma_start(out=wg2b_sb[:], in_=wg2b_d[:])
            nc.sync.dma_start(out=convw_sb[:], in_=convw_d[:])
            nc.sync.dma_start(out=maskc_sb[:], in_=maskc_d[:])
            nc.gpsimd.dma_start(out=ident[:], in_=ident_d[:])
            nc.gpsimd.dma_start(out=triu2[:], in_=triu2_d[:])
            dma_engs = [nc.sync, nc.scalar, nc.gpsimd]
            for hf in range(16):
                kt, nh = hf // 2, hf % 2
                dma_engs[hf % 3].dma_start(
                    out=srcT[:, kt, nh * 512:(nh + 1) * 512],
                    in_=srcT_d[:, kt * T + nh * 512:kt * T + (nh + 1) * 512],
                )
            nc.scalar.dma_start(out=wq_sb[:].rearrange("p a b -> p (a b)"), in_=wq_d[:])
            nc.sync.dma_start(out=wk_sb[:].rearrange("p a b -> p (a b)"), in_=wk_d[:])
            nc.gpsimd.dma_start(out=wv_sb[:].rearrange("p a b -> p (a b)"), in_=wv_d[:])
            nc.gpsimd.dma_start(out=wgate_sb[:].rearrange("p a b -> p (a b)"), in_=wgate_d[:])
            nc.gpsimd.dma_start(out=wo_sb[:].rearrange("p a b -> p (a b)"), in_=wo_d[:])

            nc.vector.memset(ones_sb[:], 1.0)
            nc.vector.memset(xgT[:], 1.0)
            nc.vector.memset(negln8[:], -LN8)
            nc.vector.memset(magic4[:], RSQRT_MAGIC)
            nc.vector.memset(eps_col[:], EPS)
            for g in range(G):
                nc.vector.memset(Sblk[g][:], 0.0)
                nc.vector.memset(qblk[g][:], 0.0)

            # ---- gk path -----------------------------------------------------
            for nh in range(2):
                p = ps_big.tile([16, 512], F32, name="pp_xg", tag="ppb")
                for kt in range(8):
                    nc.tensor.matmul(
                        p[:], wg1_sb[:, kt, :], srcT[:, kt, nh * 512:(nh + 1) * 512],
                        start=(kt == 0), stop=(kt == 7),
                    )
                nc.vector.tensor_copy(out=xgT[0:16, nh * 512:(nh + 1) * 512], in_=p[:])
            enxs = []
            for mi in range(MIQ):
                for nh in range(2):
                    p = ps_big.tile([128, 512], F32, name="pp_sp", tag="ppb")
                    nc.tensor.matmul(
                        p[:], wg2b_sb[:, mi * 128:(mi + 1) * 128],
                        xgT[:, nh * 512:(nh + 1) * 512], start=True, stop=True,
                    )
                    enx = scr2.tile([128, 512], BF, name="enx", tag="enx", bufs=4)
                    nc.scalar.activation(enx[:], p[:], AF.Exp, scale=-1.0)
                    enxs.append((mi, nh, enx))
            for mi, nh, enx in enxs:
                nc.scalar.activation(
                    spT[:, mi, nh * 512:(nh + 1) * 512], enx[:], AF.Ln, bias=1.0,
                )
            for mi in range(MIQ):
                for c in range(NCH):
                    csl = slice(c * 128, (c + 1) * 128)
                    nc.vector.tensor_tensor_scan(
                        out=bsum[:, mi, csl], data0=ones_sb[:], data1=spT[:, mi, csl],
                        initial=0.0, op0=OP.mult, op1=OP.add,
                    )
                ends = bsum[:, mi, :].rearrange("p (c s) -> p c s", s=128)[:, :, 127:128]
                bce = scr.tile([128, 8], F32, name="bce", tag="bce")
                nc.vector.tensor_copy(
                    out=bce[:].rearrange("p (c one) -> p c one", one=1), in_=ends
                )
                nc.scalar.activation(Eall[:, mi, :], bce[:], AF.Exp, scale=-1.0 / GATE_NORM)
                nc.scalar.activation(
                    texpq[:, mi, :], bsum[:, mi, :], AF.Exp,
                    scale=-1.0 / GATE_NORM, bias=negln8[:],
                )
                nc.scalar.activation(
                    texpk[:, mi, :], bsum[:, mi, :], AF.Exp, scale=1.0 / GATE_NORM,
                )

            # ---- projections; conv4 on DVE/GpSimd in fp16 -------------------
            def conv_proj(w_sb, dst, mi_count, ti_base, conv_eng_for_mi):
                for mi in range(mi_count):
                    ceng = conv_eng_for_mi(mi)
                    pre = scr2.tile([128, 1028], F16, name="pre", tag="pre")
                    nc.gpsimd.memset(pre[:, 0:4], 0.0)
                    for nh in range(2):
                        p = ps_big.tile([128, 512], F32, name="pp_c", tag="ppb")
                        for kt in range(8):
                            nc.tensor.matmul(
                                p[:], w_sb[:, kt, mi * 128:(mi + 1) * 128],
                                srcT[:, kt, nh * 512:(nh + 1) * 512],
                                start=(kt == 0), stop=(kt == 7),
                            )
                        nc.vector.tensor_copy(
                            out=pre[:, 4 + nh * 512:4 + (nh + 1) * 512], in_=p[:]
                        )
                    acc = scr2.tile([128, 1024], F16, name="acc", tag="acc")
                    for nh in range(2):
                        seg = slice(nh * 512, (nh + 1) * 512)
                        t4 = (ti_base + mi) * 4
                        nc_e = ceng
                        nc_e.tensor_scalar_mul(
                            acc[:, seg], pre[:, 1 + nh * 512:1 + nh * 512 + 512],
                            convw_sb[:, t4:t4 + 1],
                        )
                        for j in range(1, CONV):
                            nc_e.scalar_tensor_tensor(
                                out=acc[:, seg],
                                in0=pre[:, 1 + nh * 512 + j:1 + nh * 512 + j + 512],
                                scalar=convw_sb[:, t4 + j:t4 + j + 1],
                                in1=acc[:, seg], op0=OP.mult, op1=OP.add,
                            )
                        if USE_SILU:
                            nc.scalar.activation(
                                dst[:, mi, seg], acc[:, seg], AF.Silu,
                            )
                        else:
                            sg = scr2.tile([128, 512], BF, name="sg", tag="sg")
                            nc.scalar.activation(sg[:], acc[:, seg], AF.Sigmoid)
                            nc.vector.tensor_mul(dst[:, mi, seg], acc[:, seg], sg[:])

            conv_proj(wq_sb, q_sb, MIQ, 0, lambda mi: nc.vector)
            conv_proj(wk_sb, k_sb, MIQ, MIQ, lambda mi: nc.vector)
            for mi in range(MIQ):
                nc.vector.tensor_mul(q_sb[:, mi, :], q_sb[:, mi, :], texpq[:, mi, :])
                nc.vector.tensor_mul(k_sb[:, mi, :], k_sb[:, mi, :], texpk[:, mi, :])
            conv_proj(wv_sb, v_sb, MIV, 2 * MIQ, lambda mi: nc.vector)

            # ---- chunk loop: gate proj + recurrence + (lagged) fused tail ---
            def emit_gate(mt):
                p = ps_big.tile([128, 512], F32, name="pp_g", tag="ppb")
                for kt in range(8):
                    nc.tensor.matmul(
                        p[:], srcT[:, kt, mt * 128:(mt + 1) * 128], wgate_sb[:, kt, :],
                        start=(kt == 0), stop=(kt == 7),
                    )
                if USE_SILU:
                    nc.scalar.activation(gate_sb[:, mt, :], p[:], AF.Silu)
                else:
                    sgg = scr2.tile([128, 512], BF, name="sgg", tag="sg")
                    nc.scalar.activation(sgg[:], p[:], AF.Sigmoid)
                    nc.vector.tensor_mul(gate_sb[:, mt, :], p[:], sgg[:])

            def emit_recurrence(c):
                csl = slice(c * 128, (c + 1) * 128)
                front = []
                for g in range(G):
                    e_col = Eall[:, g, c:c + 1]
                    kh_s = scr.tile([128, 128], BF, name="kh_s", tag="kh_s")
                    nc.vector.tensor_scalar_mul(kh_s[:], k_sb[:, g, csl], e_col)
                    nc.vector.tensor_copy(out=qblk[g][0:64, 0:128], in_=q_sb[0:64, g, csl])
                    nc.vector.tensor_copy(out=qblk[g][64:128, 128:256], in_=q_sb[64:128, g, csl])
                    ps_a = ps_sm.tile([128, 256], F32, name="ps_a", tag="ps_sm")
                    nc.tensor.matmul(
                        ps_a[:], k_sb[:, g, csl], qblk[g][:], start=True, stop=True,
                    )
                    a_sb = scr.tile([128, 256], BF, name="a_sb", tag="a_sb")
                    nc.vector.tensor_mul(a_sb[:], ps_a[:], triu2[:])
                    ps_v = ps_sm.tile([128, 256], BF, name="ps_v", tag="ps_sm")
                    nc.tensor.matmul(
                        ps_v[:, 0:128], v_sb[:, 2 * g, csl], ident[:],
                        is_transpose=True, start=True, stop=False, skip_group_check=True,
                    )
                    nc.tensor.matmul(
                        ps_v[:, 128:256], v_sb[:, 2 * g + 1, csl], ident[:],
                        is_transpose=True, start=False, stop=True, skip_group_check=True,
                    )
                    vnat = scr.tile([128, 256], BF, name="vnat", tag="vnat")
                    nc.vector.tensor_scalar_mul(vnat[:], ps_v[:], maskc_sb[:, c:c + 1])
                    ps_k = ps_sm.tile([128, 128], BF, name="ps_k", tag="ps_sm")
                    nc.tensor.transpose(ps_k[:], kh_s[:], ident[:])
                    khnat = scr.tile([128, 128], BF, name="khnat", tag="khnat")
                    nc.vector.tensor_copy(out=khnat[:], in_=ps_k[:])
                    front.append((a_sb, vnat, khnat, e_col))
                ps_os = []
                for g in range(G):
                    a_sb, vnat, khnat, e_col = front[g]
                    ps_o = ps_op.tile([128, 256], F32, name="ps_o", tag="ps_o")
                    nc.tensor.matmul(
                        ps_o[:, 0:128], a_sb[:, 0:128], vnat[:, 0:128],
                        start=True, stop=False, skip_group_check=True,
                    )
                    nc.tensor.matmul(
                        ps_o[:, 128:256], a_sb[:, 128:256], vnat[:, 128:256],
                        start=False, stop=False, skip_group_check=True,
                    )
                    nc.tensor.matmul(
                        ps_o[:], q_sb[:, g, csl], Sblk[g][:],
                        start=False, stop=True, skip_group_check=True,
                    )
                    ps_s = ps_sm.tile([128, 256], F32, name="ps_s", tag="ps_sm")
                    nc.tensor.matmul(ps_s[:], khnat[:], vnat[:], start=True, stop=True)
                    nc.vector.scalar_tensor_tensor(
                        out=Sblk[g][0:64, 0:128], in0=Sblk[g][0:64, 0:128],
                        scalar=e_col[0:64, :], in1=ps_s[0:64, 0:128],
                        op0=OP.mult, op1=OP.add,
                    )
                    nc.vector.scalar_tensor_tensor(
                        out=Sblk[g][64:128, 128:256], in0=Sblk[g][64:128, 128:256],
                        scalar=e_col[64:128, :], in1=ps_s[64:128, 128:256],
                        op0=OP.mult, op1=OP.add,
                    )
                    idx = c * 4 + 2 * g
                    for lh in range(2):
                        sqd = scr.tile([128, 128], BF, name="sqd", tag="sqd")
                        nc.scalar.activation(
                            sqd[:], ps_o[:, lh * 128:(lh + 1) * 128], AF.Square,
                            accum_out=ssq_all[:, idx + lh:idx + lh + 1],
                        )
                    ps_os.append(ps_o)
                # rr = rsqrt(ssq/DV + eps) on DVE: bit-trick seed + 2 Newton
                cs4 = slice(c * 4, c * 4 + 4)
                tloc = nwt.tile([128, 4], F32, name="tloc", tag="nt")
                nc.vector.tensor_scalar(
                    out=tloc[:], in0=ssq_all[:, cs4], scalar1=1.0 / DV, scalar2=EPS,
                    op0=OP.mult, op1=OP.add,
                )
                sh = nwt.tile([128, 4], I32, name="sh", tag="nsh")
                nc.vector.tensor_scalar(
                    out=sh[:], in0=tloc[:].bitcast(I32), scalar1=1, scalar2=None,
                    op0=OP.logical_shift_right,
                )
                yv = nwt.tile([128, 4], F32, name="yv", tag="ny")
                nc.vector.tensor_tensor(
                    out=yv[:].bitcast(I32), in0=magic4[:], in1=sh[:], op=OP.subtract,
                )
                for _ in range(2):
                    y2 = nwt.tile([128, 4], F32, name="y2", tag="n2")
                    nc.vector.tensor_mul(y2[:], yv[:], yv[:])
                    nc.vector.tensor_mul(y2[:], y2[:], tloc[:])
                    nc.vector.tensor_scalar(
                        out=y2[:], in0=y2[:], scalar1=-0.5, scalar2=1.5,
                        op0=OP.mult, op1=OP.add,
                    )
                    yn = nwt.tile([128, 4], F32, name="yn", tag="ny")
                    nc.vector.tensor_mul(yn[:], yv[:], y2[:])
                    yv = yn
                nc.vector.tensor_copy(out=rr_all[:, cs4], in_=yv[:])
                # gated output: gate_sb <- (o * rr) * gate
                for g in range(G):
                    idx = c * 4 + 2 * g
                    for lh in range(2):
                        gdst = gate_sb[:, c, g * 256 + lh * 128:g * 256 + (lh + 1) * 128]
                        nc.vector.scalar_tensor_tensor(
                            out=gdst, in0=ps_os[g][:, lh * 128:(lh + 1) * 128],
                            scalar=rr_all[:, idx + lh:idx + lh + 1], in1=gdst,
                            op0=OP.mult, op1=OP.mult,
                        )

            def emit_tail(c):
                csl = slice(c * 128, (c + 1) * 128)
                ogs = []
                for hp in range(0, 4, 2):
                    ps_g = ps_sm.tile([128, 256], BF, name="ps_g", tag="ps_sm")
                    nc.tensor.matmul(
                        ps_g[:, 0:128], gate_sb[:, c, hp * 128:(hp + 1) * 128],
                        ident[:], is_transpose=True, start=True, stop=False,
                        skip_group_check=True,
                    )
                    nc.tensor.matmul(
                        ps_g[:, 128:256], gate_sb[:, c, (hp + 1) * 128:(hp + 2) * 128],
                        ident[:], is_transpose=True, start=False, stop=True,
                        skip_group_check=True,
                    )
                    og = scr.tile([128, 256], BF, name="og", tag="og")
                    nc.vector.tensor_copy(out=og[:], in_=ps_g[:])
                    ogs.append(og)
                for nh in range(2):
                    p = ps_big.tile([128, 512], F32, name="p_out", tag="ppb")
                    for h in range(4):
                        nc.tensor.matmul(
                            p[:], ogs[h // 2][:, (h % 2) * 128:(h % 2 + 1) * 128],
                            wo_sb[:, h, nh * 512:(nh + 1) * 512],
                            start=(h == 0), stop=(h == 3),
                        )
                    stg = stage_pool.tile([128, 512], F32, name="stage", tag="stage")
                    if nh == 0:
                        nc.vector.tensor_copy(out=stg[:], in_=p[:])
                    else:
                        nc.scalar.copy(out=stg[:], in_=p[:])
                    nc.sync.dma_start(
                        out=out_d[c * 128:(c + 1) * 128, nh * 512:(nh + 1) * 512],
                        in_=stg[:],
                    )

            for c in range(NCH):
                emit_gate(c)
                if c > 0:
                    emit_tail(c - 1)
                emit_recurrence(c)
            emit_tail(NCH - 1)

    nc.compile()
    return nc


_NC_CACHE = None


def _get_program():
    global _NC_CACHE
    if _NC_CACHE is None:
        _NC_CACHE = build_program()
    return _NC_CACHE


def shard_inputs(
    src, valid_mask, Wq, Wk, Wv, conv_q_w, conv_k_w, conv_v_w,
    Wg1, Wg2, bg2, Wgate, rms_w, Wo,
):
    bf = ml_dtypes.bfloat16
    f = np.float32

    def pack_km(w, m):
        return np.ascontiguousarray(
            np.asarray(w, f).reshape(8, 128, m).transpose(1, 0, 2).reshape(128, 8 * m)
        ).astype(bf)

    src = np.asarray(src, f)
    valid_mask = np.asarray(valid_mask)
    wo_scaled = np.asarray(Wo, f) * np.tile(np.asarray(rms_w, f), VD // DV)[:, None]
    in_maps = []
    for core in range(NCORES):
        b, hg = core // 2, core % 2
        qs = slice(hg * KDC, (hg + 1) * KDC)
        vs = slice(hg * VDC, (hg + 1) * VDC)
        wg2b = np.concatenate(
            [np.asarray(Wg2, f)[:, qs], np.asarray(bg2, f)[None, qs]], axis=0
        )
        convw = np.zeros((128, 32), f)
        ti = 0
        for w, sel, n in ((conv_q_w, qs, MIQ), (conv_k_w, qs, MIQ),
                          (conv_v_w, vs, MIV)):
            wa = np.asarray(w, f)[sel]
            for i in range(n):
                convw[:, ti * 4:(ti + 1) * 4] = wa[i * 128:(i + 1) * 128]
                ti += 1
        wo_core = np.ascontiguousarray(
            wo_scaled[vs].reshape(4, 128, 1024).transpose(1, 0, 2).reshape(128, 4096)
        ).astype(bf)
        in_maps.append({
            "srcT_in": np.ascontiguousarray(
                src[b].T.reshape(8, 128, T).transpose(1, 0, 2).reshape(128, 8 * T)
            ).astype(bf),
            "wq": pack_km(np.asarray(Wq, f)[:, qs], 256),
            "wk": pack_km(np.asarray(Wk, f)[:, qs], 256),
            "wv": pack_km(np.asarray(Wv, f)[:, vs], 512),
            "wgate": pack_km(np.asarray(Wgate, f)[:, vs], 512),
            "wg1": pack_km(np.asarray(Wg1, f), 16),
            "wg2b": np.ascontiguousarray(wg2b).astype(bf),
            "wo": wo_core,
            "convw": convw,
            "maskc": np.ascontiguousarray(
                valid_mask[b].astype(f).reshape(NCH, 128).T
            ),
        })
    return in_maps


def kernel(**inputs):
    nc = _get_program()
    in_maps = shard_inputs(**inputs)
    res = run_bass_kernel_spmd(nc, in_maps, list(range(NCORES)))
    out = np.zeros((B, T, D), np.float32)
    for core in range(NCORES):
        out[core // 2] += res.results[core]["out"]
    return out


if __name__ == "__main__":
    prog = _get_program()
    print("program built OK")


# revision 12
# speedup vs baseline: 1.3127x; 1.0912x over previous
"""Gated Linear Attention forward on 8 Trainium2 NeuronCores (Bass/Tile).

Problem: B=4, T=1024, D=1024, H=8, DK=64, DV=128, conv4 on q/k/v, low-rank
log-sigmoid forget gate, recurrent scan, RMS-norm + swish output gate, out proj.

Sharding: core = 2*b + hg  (b = batch, hg = half of the heads).
Each core computes its batch's tokens for 4 heads end-to-end and a partial
output projection (Wo row-block); the host sums the two partials per batch.

v4: bf16 matmuls (fp32 PSUM), and:
 - src arrives as 16 half-tiles issued first across all three DMA queues;
   weight DMAs are single straight descriptors (no AP splitting).
 - depthwise conv4 runs on DVE/GpSimd as shifted scalar_tensor_tensor chains
   in fp16 (frees ~20us of PE diag-matmul time).
 - silu is a single ACT op where available; the ACT table schedule is
   exp -> ln -> exp -> silu with NOTHING in the chunk loop but filler
   functions (square/copy), so zero in-loop table loads.
 - the per-chunk RMS rsqrt is computed on DVE (bit-trick seed + 2 Newton
   steps) instead of ACT Ln/Exp, which would force 4 table loads per chunk.
 - the output gate projection is interleaved chunk-by-chunk with the GLA
   recurrence, and each chunk's rms/gate/transpose/out-projection tail is
   emitted one chunk behind the recurrence so the Newton-rr latency stays
   off the critical path; output streams to DRAM throughout.
"""

import numpy as np
import ml_dtypes

import concourse.bass as bass
import concourse.mybir as mybir
import concourse.tile as tile
from concourse import bacc
from concourse.bass_utils import run_bass_kernel_spmd

F32 = mybir.dt.float32
BF = mybir.dt.bfloat16
F16 = mybir.dt.float16
I32 = mybir.dt.int32
AF = mybir.ActivationFunctionType
OP = mybir.AluOpType

B, T, D, H = 4, 1024, 1024, 8
KD, VD = 512, 1024
DK, DV = 64, 128
CONV = 4
GATE_NORM = 16.0
EPS = 1e-5
LN8 = float(np.log(8.0))

KDC, VDC = 256, 512
MIQ, MIV = 2, 4
C, NCH = 128, 8
G = 2
NCORES = 8

RSQRT_MAGIC = 0x5F3759DF
import os as _os
USE_SILU = _os.environ.get("K_NO_SILU") != "1"   # HW-verified; CoreSim lacks Silu


def build_program():
    nc = bacc.Bacc("TRN2", target_bir_lowering=False, debug=False)

    srcT_d = nc.dram_tensor("srcT_in", [128, 8 * T], BF, kind="ExternalInput")
    wq_d = nc.dram_tensor("wq", [128, 8 * 256], BF, kind="ExternalInput")
    wk_d = nc.dram_tensor("wk", [128, 8 * 256], BF, kind="ExternalInput")
    wv_d = nc.dram_tensor("wv", [128, 8 * 512], BF, kind="ExternalInput")
    wgate_d = nc.dram_tensor("wgate", [128, 8 * 512], BF, kind="ExternalInput")
    wg1_d = nc.dram_tensor("wg1", [128, 8 * 16], BF, kind="ExternalInput")
    wg2b_d = nc.dram_tensor("wg2b", [17, KDC], BF, kind="ExternalInput")
    wo_d = nc.dram_tensor("wo", [128, 4 * 1024], BF, kind="ExternalInput")
    convw_d = nc.dram_tensor("convw", [128, 32], F32, kind="ExternalInput")
    maskc_d = nc.dram_tensor("maskc", [128, NCH], F32, kind="ExternalInput")
    out_d = nc.dram_tensor("out", [T, D], F32, kind="ExternalOutput")

    ident_np = np.eye(128, dtype=ml_dtypes.bfloat16)
    u = np.triu(np.ones((128, 128), np.float32)).astype(ml_dtypes.bfloat16)
    ident_d = nc.inline_tensor(ident_np, "ident_c")
    triu2_d = nc.inline_tensor(np.concatenate([u, u], axis=1), "triu2_c")

    srcT = nc.alloc_sbuf_tensor("srcT", [128, 8, T], BF)
    q_sb = nc.alloc_sbuf_tensor("q_sb", [128, MIQ, T], BF)
    k_sb = nc.alloc_sbuf_tensor("k_sb", [128, MIQ, T], BF)
    v_sb = nc.alloc_sbuf_tensor("v_sb", [128, MIV, T], BF)
    gate_sb = nc.alloc_sbuf_tensor("gate_sb", [128, NCH, VDC], BF)
    xgT = nc.alloc_sbuf_tensor("xgT", [17, T], BF)
    spT = nc.alloc_sbuf_tensor("spT", [128, MIQ, T], F32)
    bsum = nc.alloc_sbuf_tensor("bsum", [128, MIQ, T], F32)
    texpq = nc.alloc_sbuf_tensor("texpq", [128, MIQ, T], BF)
    texpk = nc.alloc_sbuf_tensor("texpk", [128, MIQ, T], BF)
    Eall = nc.alloc_sbuf_tensor("Eall", [128, MIQ, NCH], F32)
    ssq_all = nc.alloc_sbuf_tensor("ssq_all", [128, NCH * 4], F32)
    rr_all = nc.alloc_sbuf_tensor("rr_all", [128, NCH * 4], F32)
    wq_sb = nc.alloc_sbuf_tensor("wq_sb", [128, 8, 256], BF)
    wk_sb = nc.alloc_sbuf_tensor("wk_sb", [128, 8, 256], BF)
    wv_sb = nc.alloc_sbuf_tensor("wv_sb", [128, 8, 512], BF)
    wgate_sb = nc.alloc_sbuf_tensor("wgate_sb", [128, 8, 512], BF)
    wg1_sb = nc.alloc_sbuf_tensor("wg1_sb", [128, 8, 16], BF)
    wg2b_sb = nc.alloc_sbuf_tensor("wg2b_sb", [17, KDC], BF)
    wo_sb = nc.alloc_sbuf_tensor("wo_sb", [128, 4, 1024], BF)
    convw_sb = nc.alloc_sbuf_tensor("convw_sb", [128, 32], F32)
    maskc_sb = nc.alloc_sbuf_tensor("maskc_sb", [128, NCH], F32)
    ident = nc.alloc_sbuf_tensor("ident", [128, 128], BF)
    triu2 = nc.alloc_sbuf_tensor("triu2", [128, 256], BF)
    ones_sb = nc.alloc_sbuf_tensor("ones_sb", [128, 128], F32)
    Sblk = [nc.alloc_sbuf_tensor(f"Sblk{g}", [128, 256], BF) for g in range(G)]
    qblk = [nc.alloc_sbuf_tensor(f"qblk{g}", [128, 256], BF) for g in range(G)]
    negln8 = nc.alloc_sbuf_tensor("negln8", [128, 1], F32)
    magic4 = nc.alloc_sbuf_tensor("magic4", [128, 4], I32)
    nhalf4 = nc.alloc_sbuf_tensor("nhalf4", [128, 4], F32)
    c15_4 = nc.alloc_sbuf_tensor("c15_4", [128, 4], F32)
    eps_col = nc.alloc_sbuf_tensor("eps_col", [128, 1], F32)

    with tile.TileContext(nc) as tc:
        with (
            tc.tile_pool(name="scr", bufs=4) as scr,
            tc.tile_pool(name="scr2", bufs=4) as scr2,
            tc.tile_pool(name="nwt", bufs=2) as nwt,
            tc.tile_pool(name="stage", bufs=3) as stage_pool,
            tc.tile_pool(name="ps_big", bufs=2, space="PSUM") as ps_big,
            tc.tile_pool(name="ps_sm", bufs=4, space="PSUM") as ps_sm,
            tc.tile_pool(name="ps_op", bufs=2, space="PSUM") as ps_op,
        ):
            # ---- DMA schedule: src halves first, round-robin over queues ---
            nc.sync.dma_start(out=wg1_sb[:].rearrange("p a b -> p (a b)"), in_=wg1_d[:])
            nc.sync.dma_start(out=wg2b_sb[:], in_=wg2b_d[:])
            nc.sync.dma_start(out=convw_sb[:], in_=convw_d[:])
            nc.sync.dma_start(out=maskc_sb[:], in_=maskc_d[:])
            nc.gpsimd.dma_start(out=ident[:], in_=ident_d[:])
            nc.gpsimd.dma_start(out=triu2[:], in_=triu2_d[:])
            dma_engs = [nc.sync, nc.scalar, nc.gpsimd]
            for hf in range(16):
                kt, nh = hf // 2, hf % 2
                dma_engs[hf % 3].dma_start(
                    out=srcT[:, kt, nh * 512:(nh + 1) * 512],
                    in_=srcT_d[:, kt * T + nh * 512:kt * T + (nh + 1) * 512],
                )
            nc.scalar.dma_start(out=wq_sb[:].rearrange("p a b -> p (a b)"), in_=wq_d[:])
            nc.sync.dma_start(out=wk_sb[:].rearrange("p a b -> p (a b)"), in_=wk_d[:])
            nc.gpsimd.dma_start(out=wv_sb[:].rearrange("p a b -> p (a b)"), in_=wv_d[:])
            nc.gpsimd.dma_start(out=wgate_sb[:].rearrange("p a b -> p (a b)"), in_=wgate_d[:])
            nc.gpsimd.dma_start(out=wo_sb[:].rearrange("p a b -> p (a b)"), in_=wo_d[:])

            nc.vector.memset(ones_sb[:], 1.0)
            nc.vector.memset(xgT[:], 1.0)
            nc.vector.memset(negln8[:], -LN8)
            nc.vector.memset(magic4[:], RSQRT_MAGIC)
            nc.vector.memset(nhalf4[:], -0.5)
            nc.vector.memset(c15_4[:], 1.5)
            nc.vector.memset(eps_col[:], EPS)
            for g in range(G):
                nc.vector.memset(Sblk[g][:], 0.0)
                nc.vector.memset(qblk[g][:], 0.0)

            # ---- gk path -----------------------------------------------------
            for nh in range(2):
                p = ps_big.tile([16, 512], F32, name="pp_xg", tag="ppb")
                for kt in range(8):
                    nc.tensor.matmul(
                        p[:], wg1_sb[:, kt, :], srcT[:, kt, nh * 512:(nh + 1) * 512],
                        start=(kt == 0), stop=(kt == 7),
                    )
                nc.scalar.copy(out=xgT[0:16, nh * 512:(nh + 1) * 512], in_=p[:])
            enxs = []
            for mi in range(MIQ):
                for nh in range(2):
                    p = ps_big.tile([128, 512], F32, name="pp_sp", tag="ppb")
                    nc.tensor.matmul(
                        p[:], wg2b_sb[:, mi * 128:(mi + 1) * 128],
                        xgT[:, nh * 512:(nh + 1) * 512], start=True, stop=True,
                    )
                    enx = scr2.tile([128, 512], BF, name="enx", tag="enx", bufs=4)
                    nc.scalar.activation(enx[:], p[:], AF.Exp, scale=-1.0)
                    enxs.append((mi, nh, enx))
            for mi, nh, enx in enxs:
                nc.scalar.activation(
                    spT[:, mi, nh * 512:(nh + 1) * 512], enx[:], AF.Ln, bias=1.0,
                )
            for mi in range(MIQ):
                for c in range(NCH):
                    csl = slice(c * 128, (c + 1) * 128)
                    nc.vector.tensor_tensor_scan(
                        out=bsum[:, mi, csl], data0=ones_sb[:], data1=spT[:, mi, csl],
                        initial=0.0, op0=OP.mult, op1=OP.add,
                    )
                ends = bsum[:, mi, :].rearrange("p (c s) -> p c s", s=128)[:, :, 127:128]
                bce = scr.tile([128, 8], F32, name="bce", tag="bce")
                nc.vector.tensor_copy(
                    out=bce[:].rearrange("p (c one) -> p c one", one=1), in_=ends
                )
                nc.scalar.activation(Eall[:, mi, :], bce[:], AF.Exp, scale=-1.0 / GATE_NORM)
                nc.scalar.activation(
                    texpq[:, mi, :], bsum[:, mi, :], AF.Exp,
                    scale=-1.0 / GATE_NORM, bias=negln8[:],
                )
                nc.scalar.activation(
                    texpk[:, mi, :], bsum[:, mi, :], AF.Exp, scale=1.0 / GATE_NORM,
                )

            # ---- projections; conv4 on DVE/GpSimd in fp16 -------------------
            def conv_proj(w_sb, dst, mi_count, ti_base, conv_eng_for_mi):
                for mi in range(mi_count):
                    ceng = conv_eng_for_mi(mi)
                    pre = scr2.tile([128, 1028], F16, name="pre", tag="pre")
                    nc.gpsimd.memset(pre[:, 0:4], 0.0)
                    for nh in range(2):
                        p = ps_big.tile([128, 512], F32, name="pp_c", tag="ppb")
                        for kt in range(8):
                            nc.tensor.matmul(
                                p[:], w_sb[:, kt, mi * 128:(mi + 1) * 128],
                                srcT[:, kt, nh * 512:(nh + 1) * 512],
                                start=(kt == 0), stop=(kt == 7),
                            )
                        nc.scalar.copy(
                            out=pre[:, 4 + nh * 512:4 + (nh + 1) * 512], in_=p[:]
                        )
                    acc = scr2.tile([128, 1024], F16, name="acc", tag="acc")
                    for nh in range(2):
                        seg = slice(nh * 512, (nh + 1) * 512)
                        t4 = (ti_base + mi) * 4
                        nc_e = ceng
                        nc_e.tensor_scalar_mul(
                            acc[:, seg], pre[:, 1 + nh * 512:1 + nh * 512 + 512],
                            convw_sb[:, t4:t4 + 1],
                        )
                        for j in range(1, CONV):
                            nc_e.scalar_tensor_tensor(
                                out=acc[:, seg],
                                in0=pre[:, 1 + nh * 512 + j:1 + nh * 512 + j + 512],
                                scalar=convw_sb[:, t4 + j:t4 + j + 1],
                                in1=acc[:, seg], op0=OP.mult, op1=OP.add,
                            )
                        if USE_SILU:
                            nc.scalar.activation(
                                dst[:, mi, seg], acc[:, seg], AF.Silu,
                            )
                        else:
                            sg = scr2.tile([128, 512], BF, name="sg", tag="sg")
                            nc.scalar.activation(sg[:], acc[:, seg], AF.Sigmoid)
                            nc.vector.tensor_mul(dst[:, mi, seg], acc[:, seg], sg[:])

            conv_proj(wq_sb, q_sb, MIQ, 0, lambda mi: nc.vector)
            conv_proj(wk_sb, k_sb, MIQ, MIQ, lambda mi: nc.vector)
            for mi in range(MIQ):
                nc.vector.tensor_mul(q_sb[:, mi, :], q_sb[:, mi, :], texpq[:, mi, :])
                nc.vector.tensor_mul(k_sb[:, mi, :], k_sb[:, mi, :], texpk[:, mi, :])
            conv_proj(wv_sb, v_sb, MIV, 2 * MIQ, lambda mi: nc.vector)

            # ---- chunk loop: gate proj + recurrence + (lagged) fused tail ---
            def emit_gate(mt):
                p = ps_big.tile([128, 512], F32, name="pp_g", tag="ppb")
                for kt in range(8):
                    nc.tensor.matmul(
                        p[:], srcT[:, kt, mt * 128:(mt + 1) * 128], wgate_sb[:, kt, :],
                        start=(kt == 0), stop=(kt == 7),
                    )
                if USE_SILU:
                    nc.scalar.activation(gate_sb[:, mt, :], p[:], AF.Silu)
                else:
                    sgg = scr2.tile([128, 512], BF, name="sgg", tag="sg")
                    nc.scalar.activation(sgg[:], p[:], AF.Sigmoid)
                    nc.vector.tensor_mul(gate_sb[:, mt, :], p[:], sgg[:])

            fronts = {}

            def emit_front(c):
                csl = slice(c * 128, (c + 1) * 128)
                front = []
                for g in range(G):
                    e_col = Eall[:, g, c:c + 1]
                    kh_s = scr.tile([128, 128], BF, name="kh_s", tag="kh_s")
                    nc.vector.tensor_scalar_mul(kh_s[:], k_sb[:, g, csl], e_col)
                    nc.gpsimd.tensor_copy(out=qblk[g][0:64, 0:128], in_=q_sb[0:64, g, csl])
                    nc.gpsimd.tensor_copy(out=qblk[g][64:128, 128:256], in_=q_sb[64:128, g, csl])
                    ps_a = ps_sm.tile([128, 256], F32, name="ps_a", tag="ps_sm")
                    nc.tensor.matmul(
                        ps_a[:], k_sb[:, g, csl], qblk[g][:], start=True, stop=True,
                    )
                    a_sb = scr.tile([128, 256], BF, name="a_sb", tag="a_sb")
                    nc.vector.tensor_mul(a_sb[:], ps_a[:], triu2[:])
                    ps_v = ps_sm.tile([128, 256], BF, name="ps_v", tag="ps_sm")
                    nc.tensor.matmul(
                        ps_v[:, 0:128], v_sb[:, 2 * g, csl], ident[:],
                        is_transpose=True, start=True, stop=False, skip_group_check=True,
                    )
                    nc.tensor.matmul(
                        ps_v[:, 128:256], v_sb[:, 2 * g + 1, csl], ident[:],
                        is_transpose=True, start=False, stop=True, skip_group_check=True,
                    )
                    vnat = scr.tile([128, 256], BF, name="vnat", tag="vnat")
                    nc.vector.tensor_scalar_mul(vnat[:], ps_v[:], maskc_sb[:, c:c + 1])
                    ps_k = ps_sm.tile([128, 128], BF, name="ps_k", tag="ps_sm")
                    nc.tensor.transpose(ps_k[:], kh_s[:], ident[:])
                    khnat = scr.tile([128, 128], BF, name="khnat", tag="khnat")
                    nc.vector.tensor_copy(out=khnat[:], in_=ps_k[:])
                    front.append((a_sb, vnat, khnat, e_col))
                fronts[c] = front

            def emit_back(c):
                csl = slice(c * 128, (c + 1) * 128)
                front = fronts.pop(c)
                ps_os = []
                for g in range(G):
                    a_sb, vnat, khnat, e_col = front[g]
                    ps_o = ps_op.tile([128, 256], F32, name="ps_o", tag="ps_o")
                    nc.tensor.matmul(
                        ps_o[:, 0:128], a_sb[:, 0:128], vnat[:, 0:128],
                        start=True, stop=False, skip_group_check=True,
                    )
                    nc.tensor.matmul(
                        ps_o[:, 128:256], a_sb[:, 128:256], vnat[:, 128:256],
                        start=False, stop=False, skip_group_check=True,
                    )
                    nc.tensor.matmul(
                        ps_o[:], q_sb[:, g, csl], Sblk[g][:],
                        start=False, stop=True, skip_group_check=True,
                    )
                    ps_s = ps_sm.tile([128, 256], F32, name="ps_s", tag="ps_sm")
                    nc.tensor.matmul(ps_s[:], khnat[:], vnat[:], start=True, stop=True)
                    nc.vector.scalar_tensor_tensor(
                        out=Sblk[g][0:64, 0:128], in0=Sblk[g][0:64, 0:128],
                        scalar=e_col[0:64, :], in1=ps_s[0:64, 0:128],
                        op0=OP.mult, op1=OP.add,
                    )
                    nc.vector.scalar_tensor_tensor(
                        out=Sblk[g][64:128, 128:256], in0=Sblk[g][64:128, 128:256],
                        scalar=e_col[64:128, :], in1=ps_s[64:128, 128:256],
                        op0=OP.mult, op1=OP.add,
                    )
                    idx = c * 4 + 2 * g
                    for lh in range(2):
                        sqd = scr.tile([128, 128], BF, name="sqd", tag="sqd")
                        nc.scalar.activation(
                            sqd[:], ps_o[:, lh * 128:(lh + 1) * 128], AF.Square,
                            accum_out=ssq_all[:, idx + lh:idx + lh + 1],
                        )
                    ps_os.append(ps_o)
                # rr = rsqrt(ssq/DV + eps) on DVE: bit-trick seed + 2 Newton
                cs4 = slice(c * 4, c * 4 + 4)
                tloc = nwt.tile([128, 4], F32, name="tloc", tag="nt")
                nc.vector.tensor_scalar(
                    out=tloc[:], in0=ssq_all[:, cs4], scalar1=1.0 / DV, scalar2=EPS,
                    op0=OP.mult, op1=OP.add,
                )
                sh = nwt.tile([128, 4], I32, name="sh", tag="nsh")
                nc.vector.tensor_scalar(
                    out=sh[:], in0=tloc[:].bitcast(I32), scalar1=1, scalar2=None,
                    op0=OP.logical_shift_right,
                )
                yv = nwt.tile([128, 4], F32, name="yv", tag="ny")
                nc.gpsimd.tensor_tensor(
                    out=yv[:].bitcast(I32), in0=magic4[:], in1=sh[:], op=OP.subtract,
                )
                for _ in range(2):
                    y2 = nwt.tile([128, 4], F32, name="y2", tag="n2")
                    nc.gpsimd.tensor_mul(y2[:], yv[:], yv[:])
                    nc.gpsimd.tensor_mul(y2[:], y2[:], tloc[:])
                    nc.gpsimd.tensor_mul(y2[:], y2[:], nhalf4[:])
                    nc.gpsimd.tensor_tensor(
                        out=y2[:], in0=y2[:], in1=c15_4[:], op=OP.add,
                    )
                    yn = nwt.tile([128, 4], F32, name="yn", tag="ny")
                    nc.gpsimd.tensor_mul(yn[:], yv[:], y2[:])
                    yv = yn
                nc.gpsimd.tensor_copy(out=rr_all[:, cs4], in_=yv[:])
                # gated output: gate_sb <- (o * rr) * gate
                for g in range(G):
                    idx = c * 4 + 2 * g
                    for lh in range(2):
                        gdst = gate_sb[:, c, g * 256 + lh * 128:g * 256 + (lh + 1) * 128]
                        nc.vector.scalar_tensor_tensor(
                            out=gdst, in0=ps_os[g][:, lh * 128:(lh + 1) * 128],
                            scalar=rr_all[:, idx + lh:idx + lh + 1], in1=gdst,
                            op0=OP.mult, op1=OP.mult,
                        )

            def emit_tail(c):
                csl = slice(c * 128, (c + 1) * 128)
                ogs = []
                for hp in range(0, 4, 2):
                    ps_g = ps_sm.tile([128, 256], BF, name="ps_g", tag="ps_sm")
                    nc.tensor.matmul(
                        ps_g[:, 0:128], gate_sb[:, c, hp * 128:(hp + 1) * 128],
                        ident[:], is_transpose=True, start=True, stop=False,
                        skip_group_check=True,
                    )
                    nc.tensor.matmul(
                        ps_g[:, 128:256], gate_sb[:, c, (hp + 1) * 128:(hp + 2) * 128],
                        ident[:], is_transpose=True, start=False, stop=True,
                        skip_group_check=True,
                    )
                    og = scr.tile([128, 256], BF, name="og", tag="og")
                    nc.vector.tensor_copy(out=og[:], in_=ps_g[:])
                    ogs.append(og)
                for nh in range(2):
                    p = ps_big.tile([128, 512], F32, name="p_out", tag="ppb")
                    for h in range(4):
                        nc.tensor.matmul(
                            p[:], ogs[h // 2][:, (h % 2) * 128:(h % 2 + 1) * 128],
                            wo_sb[:, h, nh * 512:(nh + 1) * 512],
                            start=(h == 0), stop=(h == 3),
                        )
                    stg = stage_pool.tile([128, 512], F32, name="stage", tag="stage")
                    if nh == 0:
                        nc.vector.tensor_copy(out=stg[:], in_=p[:])
                    else:
                        nc.scalar.copy(out=stg[:], in_=p[:])
                    nc.sync.dma_start(
                        out=out_d[c * 128:(c + 1) * 128, nh * 512:(nh + 1) * 512],
                        in_=stg[:],
                    )

            for c in range(NCH):
                emit_gate(c)
                emit_front(c)
                if c > 0:
                    emit_back(c - 1)
                if c > 1:
                    emit_tail(c - 2)
            emit_back(NCH - 1)
            emit_tail(NCH - 2)
            emit_tail(NCH - 1)

    nc.compile()
    return nc


_NC_CACHE = None


def _get_program():
    global _NC_CACHE
    if _NC_CACHE is None:
        _NC_CACHE = build_program()
    return _NC_CACHE


def shard_inputs(
    src, valid_mask, Wq, Wk, Wv, conv_q_w, conv_k_w, conv_v_w,
    Wg1, Wg2, bg2, Wgate, rms_w, Wo,
):
    bf = ml_dtypes.bfloat16
    f = np.float32

    def pack_km(w, m):
        return np.ascontiguousarray(
            np.asarray(w, f).reshape(8, 128, m).transpose(1, 0, 2).reshape(128, 8 * m)
        ).astype(bf)

    src = np.asarray(src, f)
    valid_mask = np.asarray(valid_mask)
    wo_scaled = np.asarray(Wo, f) * np.tile(np.asarray(rms_w, f), VD // DV)[:, None]
    in_maps = []
    for core in range(NCORES):
        b, hg = core // 2, core % 2
        qs = slice(hg * KDC, (hg + 1) * KDC)
        vs = slice(hg * VDC, (hg + 1) * VDC)
        wg2b = np.concatenate(
            [np.asarray(Wg2, f)[:, qs], np.asarray(bg2, f)[None, qs]], axis=0
        )
        convw = np.zeros((128, 32), f)
        ti = 0
        for w, sel, n in ((conv_q_w, qs, MIQ), (conv_k_w, qs, MIQ),
                          (conv_v_w, vs, MIV)):
            wa = np.asarray(w, f)[sel]
            for i in range(n):
                convw[:, ti * 4:(ti + 1) * 4] = wa[i * 128:(i + 1) * 128]
                ti += 1
        wo_core = np.ascontiguousarray(
            wo_scaled[vs].reshape(4, 128, 1024).transpose(1, 0, 2).reshape(128, 4096)
        ).astype(bf)
        in_maps.append({
            "srcT_in": np.ascontiguousarray(
                src[b].T.reshape(8, 128, T).transpose(1, 0, 2).reshape(128, 8 * T)
            ).astype(bf),
            "wq": pack_km(np.asarray(Wq, f)[:, qs], 256),
            "wk": pack_km(np.asarray(Wk, f)[:, qs], 256),
            "wv": pack_km(np.asarray(Wv, f)[:, vs], 512),
            "wgate": pack_km(np.asarray(Wgate, f)[:, vs], 512),
            "wg1": pack_km(np.asarray(Wg1, f), 16),
            "wg2b": np.ascontiguousarray(wg2b).astype(bf),
            "wo": wo_core,
            "convw": convw,
            "maskc": np.ascontiguousarray(
                valid_mask[b].astype(f).reshape(NCH, 128).T
            ),
        })
    return in_maps


def kernel(**inputs):
    nc = _get_program()
    in_maps = shard_inputs(**inputs)
    res = run_bass_kernel_spmd(nc, in_maps, list(range(NCORES)))
    out = np.zeros((B, T, D), np.float32)
    for core in range(NCORES):
        out[core // 2] += res.results[core]["out"]
    return out


if __name__ == "__main__":
    prog = _get_program()
    print("program built OK")


# revision 14
# speedup vs baseline: 1.3599x; 1.0359x over previous
"""Gated Linear Attention forward on 8 Trainium2 NeuronCores (Bass/Tile).

Problem: B=4, T=1024, D=1024, H=8, DK=64, DV=128, conv4 on q/k/v, low-rank
log-sigmoid forget gate, recurrent scan, RMS-norm + swish output gate, out proj.

Sharding: core = 2*b + hg  (b = batch, hg = half of the heads).
Each core computes its batch's tokens for 4 heads end-to-end and a partial
output projection (Wo row-block); the host sums the two partials per batch.

v4: bf16 matmuls (fp32 PSUM), and:
 - src arrives as 16 half-tiles issued first across all three DMA queues;
   weight DMAs are single straight descriptors (no AP splitting).
 - depthwise conv4 runs on DVE/GpSimd as shifted scalar_tensor_tensor chains
   in fp16 (frees ~20us of PE diag-matmul time).
 - silu is a single ACT op where available; the ACT table schedule is
   exp -> ln -> exp -> silu with NOTHING in the chunk loop but filler
   functions (square/copy), so zero in-loop table loads.
 - the per-chunk RMS rsqrt is computed on DVE (bit-trick seed + 2 Newton
   steps) instead of ACT Ln/Exp, which would force 4 table loads per chunk.
 - the output gate projection is interleaved chunk-by-chunk with the GLA
   recurrence, and each chunk's rms/gate/transpose/out-projection tail is
   emitted one chunk behind the recurrence so the Newton-rr latency stays
   off the critical path; output streams to DRAM throughout.
"""

import numpy as np
import ml_dtypes

import concourse.bass as bass
import concourse.mybir as mybir
import concourse.tile as tile
from concourse import bacc
from concourse.bass_utils import run_bass_kernel_spmd

F32 = mybir.dt.float32
BF = mybir.dt.bfloat16
F16 = mybir.dt.float16
I32 = mybir.dt.int32
AF = mybir.ActivationFunctionType
OP = mybir.AluOpType

B, T, D, H = 4, 1024, 1024, 8
KD, VD = 512, 1024
DK, DV = 64, 128
CONV = 4
GATE_NORM = 16.0
EPS = 1e-5
LN8 = float(np.log(8.0))

KDC, VDC = 256, 512
MIQ, MIV = 2, 4
C, NCH = 128, 8
G = 2
NCORES = 8

RSQRT_MAGIC = 0x5F3759DF
import os as _os
USE_SILU = _os.environ.get("K_NO_SILU") != "1"   # HW-verified; CoreSim lacks Silu


def build_program():
    nc = bacc.Bacc("TRN2", target_bir_lowering=False, debug=False)

    srcT_d = nc.dram_tensor("srcT_in", [128, 8 * T], BF, kind="ExternalInput")
    wq_d = nc.dram_tensor("wq", [128, 8 * 256], BF, kind="ExternalInput")
    wk_d = nc.dram_tensor("wk", [128, 8 * 256], BF, kind="ExternalInput")
    wv_d = nc.dram_tensor("wv", [128, 8 * 512], BF, kind="ExternalInput")
    wgate_d = nc.dram_tensor("wgate", [128, 8 * 512], BF, kind="ExternalInput")
    wg1_d = nc.dram_tensor("wg1", [128, 8 * 16], BF, kind="ExternalInput")
    wg2b_d = nc.dram_tensor("wg2b", [17, KDC], BF, kind="ExternalInput")
    wo_d = nc.dram_tensor("wo", [128, 4 * 1024], BF, kind="ExternalInput")
    convw_d = nc.dram_tensor("convw", [128, 32], F32, kind="ExternalInput")
    maskc_d = nc.dram_tensor("maskc", [128, NCH], F32, kind="ExternalInput")
    out_d = nc.dram_tensor("out", [T, D], F32, kind="ExternalOutput")

    ident_np = np.eye(128, dtype=ml_dtypes.bfloat16)
    u = np.triu(np.ones((128, 128), np.float32)).astype(ml_dtypes.bfloat16)
    ident_d = nc.inline_tensor(ident_np, "ident_c")
    triu2_d = nc.inline_tensor(np.concatenate([u, u], axis=1), "triu2_c")

    srcT = nc.alloc_sbuf_tensor("srcT", [128, 8, T], BF)
    q_sb = nc.alloc_sbuf_tensor("q_sb", [128, MIQ, T], BF)
    k_sb = nc.alloc_sbuf_tensor("k_sb", [128, MIQ, T], BF)
    v_sb = nc.alloc_sbuf_tensor("v_sb", [128, MIV, T], BF)
    gate_sb = nc.alloc_sbuf_tensor("gate_sb", [128, NCH, VDC], BF)
    xgT = nc.alloc_sbuf_tensor("xgT", [17, T], BF)
    spT = nc.alloc_sbuf_tensor("spT", [128, MIQ, T], F32)
    bsum = nc.alloc_sbuf_tensor("bsum", [128, MIQ, T], F32)
    texpq = nc.alloc_sbuf_tensor("texpq", [128, MIQ, T], BF)
    texpk = nc.alloc_sbuf_tensor("texpk", [128, MIQ, T], BF)
    Eall = nc.alloc_sbuf_tensor("Eall", [128, MIQ, NCH], F32)
    ssq_all = nc.alloc_sbuf_tensor("ssq_all", [128, NCH * 4], F32)
    rr_all = nc.alloc_sbuf_tensor("rr_all", [128, NCH * 4], F32)
    wq_sb = nc.alloc_sbuf_tensor("wq_sb", [128, 8, 256], BF)
    wk_sb = nc.alloc_sbuf_tensor("wk_sb", [128, 8, 256], BF)
    wv_sb = nc.alloc_sbuf_tensor("wv_sb", [128, 8, 512], BF)
    wgate_sb = nc.alloc_sbuf_tensor("wgate_sb", [128, 8, 512], BF)
    wg1_sb = nc.alloc_sbuf_tensor("wg1_sb", [128, 8, 16], BF)
    wg2b_sb = nc.alloc_sbuf_tensor("wg2b_sb", [17, KDC], BF)
    wo_sb = nc.alloc_sbuf_tensor("wo_sb", [128, 4, 1024], BF)
    convw_sb = nc.alloc_sbuf_tensor("convw_sb", [128, 32], F32)
    maskc_sb = nc.alloc_sbuf_tensor("maskc_sb", [128, NCH], F32)
    ident = nc.alloc_sbuf_tensor("ident", [128, 128], BF)
    triu2 = nc.alloc_sbuf_tensor("triu2", [128, 256], BF)
    ones_sb = nc.alloc_sbuf_tensor("ones_sb", [128, 128], F32)
    Sblk = [nc.alloc_sbuf_tensor(f"Sblk{g}", [128, 256], BF) for g in range(G)]
    qblk = [nc.alloc_sbuf_tensor(f"qblk{g}", [128, 256], BF) for g in range(G)]
    negln8 = nc.alloc_sbuf_tensor("negln8", [128, 1], F32)
    magic4 = nc.alloc_sbuf_tensor("magic4", [128, 4], I32)
    nhalf4 = nc.alloc_sbuf_tensor("nhalf4", [128, 4], F32)
    c15_4 = nc.alloc_sbuf_tensor("c15_4", [128, 4], F32)
    eps_col = nc.alloc_sbuf_tensor("eps_col", [128, 1], F32)

    with tile.TileContext(nc) as tc:
        with (
            tc.tile_pool(name="scr", bufs=4) as scr,
            tc.tile_pool(name="scr2", bufs=4) as scr2,
            tc.tile_pool(name="nwt", bufs=2) as nwt,
            tc.tile_pool(name="stage", bufs=3) as stage_pool,
            tc.tile_pool(name="ps_big", bufs=3, space="PSUM") as ps_big,
            tc.tile_pool(name="ps_sm", bufs=3, space="PSUM") as ps_sm,
            tc.tile_pool(name="ps_op", bufs=2, space="PSUM") as ps_op,
        ):
            # ---- DMA schedule: src halves first, round-robin over queues ---
            nc.sync.dma_start(out=wg1_sb[:].rearrange("p a b -> p (a b)"), in_=wg1_d[:])
            dma_engs = [nc.sync, nc.scalar, nc.gpsimd]
            for hf in range(16):
                kt, nh = hf // 2, hf % 2
                dma_engs[hf % 3].dma_start(
                    out=srcT[:, kt, nh * 512:(nh + 1) * 512],
                    in_=srcT_d[:, kt * T + nh * 512:kt * T + (nh + 1) * 512],
                )
            nc.sync.dma_start(out=wg2b_sb[:], in_=wg2b_d[:])
            nc.scalar.dma_start(out=convw_sb[:], in_=convw_d[:])
            nc.scalar.dma_start(out=maskc_sb[:], in_=maskc_d[:])
            nc.gpsimd.dma_start(out=ident[:], in_=ident_d[:])
            nc.gpsimd.dma_start(out=triu2[:], in_=triu2_d[:])
            nc.scalar.dma_start(out=wq_sb[:].rearrange("p a b -> p (a b)"), in_=wq_d[:])
            nc.sync.dma_start(out=wk_sb[:].rearrange("p a b -> p (a b)"), in_=wk_d[:])
            nc.gpsimd.dma_start(out=wv_sb[:].rearrange("p a b -> p (a b)"), in_=wv_d[:])
            nc.gpsimd.dma_start(out=wgate_sb[:].rearrange("p a b -> p (a b)"), in_=wgate_d[:])
            nc.gpsimd.dma_start(out=wo_sb[:].rearrange("p a b -> p (a b)"), in_=wo_d[:])

            nc.vector.memset(ones_sb[:], 1.0)
            nc.vector.memset(xgT[:], 1.0)
            nc.vector.memset(negln8[:], -LN8)
            nc.vector.memset(magic4[:], RSQRT_MAGIC)
            nc.vector.memset(nhalf4[:], -0.5)
            nc.vector.memset(c15_4[:], 1.5)
            nc.vector.memset(eps_col[:], EPS)
            for g in range(G):
                nc.vector.memset(Sblk[g][:], 0.0)
                nc.vector.memset(qblk[g][:], 0.0)

            # ---- gk path -----------------------------------------------------
            for nh in range(2):
                p = ps_big.tile([16, 512], F32, name="pp_xg", tag="ppb")
                for kt in range(8):
                    nc.tensor.matmul(
                        p[:], wg1_sb[:, kt, :], srcT[:, kt, nh * 512:(nh + 1) * 512],
                        start=(kt == 0), stop=(kt == 7),
                    )
                nc.scalar.copy(out=xgT[0:16, nh * 512:(nh + 1) * 512], in_=p[:])
            enxs = []
            for mi in range(MIQ):
                for nh in range(2):
                    p = ps_big.tile([128, 512], F32, name="pp_sp", tag="ppb")
                    nc.tensor.matmul(
                        p[:], wg2b_sb[:, mi * 128:(mi + 1) * 128],
                        xgT[:, nh * 512:(nh + 1) * 512], start=True, stop=True,
                    )
                    enx = scr2.tile([128, 512], BF, name="enx", tag="enx", bufs=4)
                    nc.scalar.activation(enx[:], p[:], AF.Exp, scale=-1.0)
                    enxs.append((mi, nh, enx))
            for mi, nh, enx in enxs:
                nc.scalar.activation(
                    spT[:, mi, nh * 512:(nh + 1) * 512], enx[:], AF.Ln, bias=1.0,
                )
            # ---- projections; conv4 on DVE/GpSimd in fp16 -------------------
            def conv_proj(w_sb, dst, mi_count, ti_base, conv_eng_for_mi,
                          evac_act=False):
                for mi in range(mi_count):
                    ceng = conv_eng_for_mi(mi)
                    pre = scr2.tile([128, 1028], F16, name="pre", tag="pre")
                    nc.gpsimd.memset(pre[:, 0:4], 0.0)
                    for nh in range(2):
                        p = ps_big.tile([128, 512], F32, name="pp_c", tag="ppb")
                        for kt in range(8):
                            nc.tensor.matmul(
                                p[:], w_sb[:, kt, mi * 128:(mi + 1) * 128],
                                srcT[:, kt, nh * 512:(nh + 1) * 512],
                                start=(kt == 0), stop=(kt == 7),
                            )
                        if evac_act:
                            nc.scalar.copy(
                                out=pre[:, 4 + nh * 512:4 + (nh + 1) * 512], in_=p[:]
                            )
                        else:
                            nc.vector.tensor_copy(
                                out=pre[:, 4 + nh * 512:4 + (nh + 1) * 512], in_=p[:]
                            )
                    acc = scr2.tile([128, 1024], F16, name="acc", tag="acc")
                    for nh in range(2):
                        seg = slice(nh * 512, (nh + 1) * 512)
                        t4 = (ti_base + mi) * 4
                        nc_e = ceng
                        nc_e.tensor_scalar_mul(
                            acc[:, seg], pre[:, 1 + nh * 512:1 + nh * 512 + 512],
                            convw_sb[:, t4:t4 + 1],
                        )
                        for j in range(1, CONV):
                            nc_e.scalar_tensor_tensor(
                                out=acc[:, seg],
                                in0=pre[:, 1 + nh * 512 + j:1 + nh * 512 + j + 512],
                                scalar=convw_sb[:, t4 + j:t4 + j + 1],
                                in1=acc[:, seg], op0=OP.mult, op1=OP.add,
                            )
                        if USE_SILU:
                            nc.scalar.activation(
                                dst[:, mi, seg], acc[:, seg], AF.Silu,
                            )
                        else:
                            sg = scr2.tile([128, 512], BF, name="sg", tag="sg")
                            nc.scalar.activation(sg[:], acc[:, seg], AF.Sigmoid)
                            nc.vector.tensor_mul(dst[:, mi, seg], acc[:, seg], sg[:])

            conv_proj(wq_sb, q_sb, MIQ, 0, lambda mi: nc.vector)
            conv_proj(wk_sb, k_sb, MIQ, MIQ, lambda mi: nc.vector)
            for mi in range(MIQ):
                for c in range(NCH):
                    csl = slice(c * 128, (c + 1) * 128)
                    nc.vector.tensor_tensor_scan(
                        out=bsum[:, mi, csl], data0=ones_sb[:], data1=spT[:, mi, csl],
                        initial=0.0, op0=OP.mult, op1=OP.add,
                    )
                ends = bsum[:, mi, :].rearrange("p (c s) -> p c s", s=128)[:, :, 127:128]
                bce = scr.tile([128, 8], F32, name="bce", tag="bce")
                nc.vector.tensor_copy(
                    out=bce[:].rearrange("p (c one) -> p c one", one=1), in_=ends
                )
                nc.scalar.activation(Eall[:, mi, :], bce[:], AF.Exp, scale=-1.0 / GATE_NORM)
                nc.scalar.activation(
                    texpq[:, mi, :], bsum[:, mi, :], AF.Exp,
                    scale=-1.0 / GATE_NORM, bias=negln8[:],
                )
                nc.scalar.activation(
                    texpk[:, mi, :], bsum[:, mi, :], AF.Exp, scale=1.0 / GATE_NORM,
                )

            for mi in range(MIQ):
                nc.vector.tensor_mul(q_sb[:, mi, :], q_sb[:, mi, :], texpq[:, mi, :])
                nc.vector.tensor_mul(k_sb[:, mi, :], k_sb[:, mi, :], texpk[:, mi, :])
            conv_proj(wv_sb, v_sb, MIV, 2 * MIQ, lambda mi: nc.vector, evac_act=True)

            # ---- chunk loop: gate proj + recurrence + (lagged) fused tail ---
            def emit_gate(mt):
                p = ps_big.tile([128, 512], F32, name="pp_g", tag="ppb")
                for kt in range(8):
                    nc.tensor.matmul(
                        p[:], srcT[:, kt, mt * 128:(mt + 1) * 128], wgate_sb[:, kt, :],
                        start=(kt == 0), stop=(kt == 7),
                    )
                if USE_SILU:
                    nc.scalar.activation(gate_sb[:, mt, :], p[:], AF.Silu)
                else:
                    sgg = scr2.tile([128, 512], BF, name="sgg", tag="sg")
                    nc.scalar.activation(sgg[:], p[:], AF.Sigmoid)
                    nc.vector.tensor_mul(gate_sb[:, mt, :], p[:], sgg[:])

            fronts = {}

            def emit_front(c):
                csl = slice(c * 128, (c + 1) * 128)
                front = []
                for g in range(G):
                    e_col = Eall[:, g, c:c + 1]
                    kh_s = scr.tile([128, 128], BF, name="kh_s", tag="kh_s")
                    nc.vector.tensor_scalar_mul(kh_s[:], k_sb[:, g, csl], e_col)
                    nc.vector.tensor_copy(out=qblk[g][0:64, 0:128], in_=q_sb[0:64, g, csl])
                    nc.vector.tensor_copy(out=qblk[g][64:128, 128:256], in_=q_sb[64:128, g, csl])
                    ps_a = ps_sm.tile([128, 256], F32, name="ps_a", tag="ps_sm")
                    nc.tensor.matmul(
                        ps_a[:], k_sb[:, g, csl], qblk[g][:], start=True, stop=True,
                    )
                    a_sb = scr.tile([128, 256], BF, name="a_sb", tag="a_sb")
                    nc.vector.tensor_mul(a_sb[:], ps_a[:], triu2[:])
                    ps_v = ps_sm.tile([128, 256], BF, name="ps_v", tag="ps_sm")
                    nc.tensor.matmul(
                        ps_v[:, 0:128], v_sb[:, 2 * g, csl], ident[:],
                        is_transpose=True, start=True, stop=False, skip_group_check=True,
                    )
                    nc.tensor.matmul(
                        ps_v[:, 128:256], v_sb[:, 2 * g + 1, csl], ident[:],
                        is_transpose=True, start=False, stop=True, skip_group_check=True,
                    )
                    vnat = scr.tile([128, 256], BF, name="vnat", tag="vnat")
                    nc.vector.tensor_scalar_mul(vnat[:], ps_v[:], maskc_sb[:, c:c + 1])
                    ps_k = ps_sm.tile([128, 128], BF, name="ps_k", tag="ps_sm")
                    nc.tensor.transpose(ps_k[:], kh_s[:], ident[:])
                    khnat = scr.tile([128, 128], BF, name="khnat", tag="khnat")
                    nc.vector.tensor_copy(out=khnat[:], in_=ps_k[:])
                    front.append((a_sb, vnat, khnat, e_col))
                fronts[c] = front

            def emit_back(c):
                csl = slice(c * 128, (c + 1) * 128)
                front = fronts.pop(c)
                ps_os = []
                for g in range(G):
                    a_sb, vnat, khnat, e_col = front[g]
                    ps_o = ps_op.tile([128, 256], F32, name="ps_o", tag="ps_o")
                    nc.tensor.matmul(
                        ps_o[:, 0:128], a_sb[:, 0:128], vnat[:, 0:128],
                        start=True, stop=False, skip_group_check=True,
                    )
                    nc.tensor.matmul(
                        ps_o[:, 128:256], a_sb[:, 128:256], vnat[:, 128:256],
                        start=False, stop=False, skip_group_check=True,
                    )
                    nc.tensor.matmul(
                        ps_o[:], q_sb[:, g, csl], Sblk[g][:],
                        start=False, stop=True, skip_group_check=True,
                    )
                    ps_s = ps_sm.tile([128, 256], F32, name="ps_s", tag="ps_sm")
                    nc.tensor.matmul(ps_s[:], khnat[:], vnat[:], start=True, stop=True)
                    nc.vector.scalar_tensor_tensor(
                        out=Sblk[g][0:64, 0:128], in0=Sblk[g][0:64, 0:128],
                        scalar=e_col[0:64, :], in1=ps_s[0:64, 0:128],
                        op0=OP.mult, op1=OP.add,
                    )
                    nc.vector.scalar_tensor_tensor(
                        out=Sblk[g][64:128, 128:256], in0=Sblk[g][64:128, 128:256],
                        scalar=e_col[64:128, :], in1=ps_s[64:128, 128:256],
                        op0=OP.mult, op1=OP.add,
                    )
                    idx = c * 4 + 2 * g
                    for lh in range(2):
                        sqd = scr.tile([128, 128], BF, name="sqd", tag="sqd")
                        nc.scalar.activation(
                            sqd[:], ps_o[:, lh * 128:(lh + 1) * 128], AF.Square,
                            accum_out=ssq_all[:, idx + lh:idx + lh + 1],
                        )
                    ps_os.append(ps_o)
                # rr = rsqrt(ssq/DV + eps) on DVE: bit-trick seed + 2 Newton
                cs4 = slice(c * 4, c * 4 + 4)
                tloc = nwt.tile([128, 4], F32, name="tloc", tag="nt")
                nc.vector.tensor_scalar(
                    out=tloc[:], in0=ssq_all[:, cs4], scalar1=1.0 / DV, scalar2=EPS,
                    op0=OP.mult, op1=OP.add,
                )
                sh = nwt.tile([128, 4], I32, name="sh", tag="nsh")
                nc.vector.tensor_scalar(
                    out=sh[:], in0=tloc[:].bitcast(I32), scalar1=1, scalar2=None,
                    op0=OP.logical_shift_right,
                )
                yv = nwt.tile([128, 4], F32, name="yv", tag="ny")
                nc.gpsimd.tensor_tensor(
                    out=yv[:].bitcast(I32), in0=magic4[:], in1=sh[:], op=OP.subtract,
                )
                for _ in range(2):
                    y2 = nwt.tile([128, 4], F32, name="y2", tag="n2")
                    nc.gpsimd.tensor_mul(y2[:], yv[:], yv[:])
                    nc.gpsimd.tensor_mul(y2[:], y2[:], tloc[:])
                    nc.gpsimd.tensor_mul(y2[:], y2[:], nhalf4[:])
                    nc.gpsimd.tensor_tensor(
                        out=y2[:], in0=y2[:], in1=c15_4[:], op=OP.add,
                    )
                    yn = nwt.tile([128, 4], F32, name="yn", tag="ny")
                    nc.gpsimd.tensor_mul(yn[:], yv[:], y2[:])
                    yv = yn
                nc.gpsimd.tensor_copy(out=rr_all[:, cs4], in_=yv[:])
                # gated output: gate_sb <- (o * rr) * gate
                for g in range(G):
                    idx = c * 4 + 2 * g
                    for lh in range(2):
                        gdst = gate_sb[:, c, g * 256 + lh * 128:g * 256 + (lh + 1) * 128]
                        nc.vector.scalar_tensor_tensor(
                            out=gdst, in0=ps_os[g][:, lh * 128:(lh + 1) * 128],
                            scalar=rr_all[:, idx + lh:idx + lh + 1], in1=gdst,
                            op0=OP.mult, op1=OP.mult,
                        )

            def emit_tail(c):
                csl = slice(c * 128, (c + 1) * 128)
                ogs = []
                for hp in range(0, 4, 2):
                    ps_g = ps_sm.tile([128, 256], BF, name="ps_g", tag="ps_sm")
                    nc.tensor.matmul(
                        ps_g[:, 0:128], gate_sb[:, c, hp * 128:(hp + 1) * 128],
                        ident[:], is_transpose=True, start=True, stop=False,
                        skip_group_check=True,
                    )
                    nc.tensor.matmul(
                        ps_g[:, 128:256], gate_sb[:, c, (hp + 1) * 128:(hp + 2) * 128],
                        ident[:], is_transpose=True, start=False, stop=True,
                        skip_group_check=True,
                    )
                    og = scr.tile([128, 256], BF, name="og", tag="og")
                    nc.vector.tensor_copy(out=og[:], in_=ps_g[:])
                    ogs.append(og)
                for nh in range(2):
                    p = ps_big.tile([128, 512], F32, name="p_out", tag="ppb")
                    for h in range(4):
                        nc.tensor.matmul(
                            p[:], ogs[h // 2][:, (h % 2) * 128:(h % 2 + 1) * 128],
                            wo_sb[:, h, nh * 512:(nh + 1) * 512],
                            start=(h == 0), stop=(h == 3),
                        )
                    stg = stage_pool.tile([128, 512], F32, name="stage", tag="stage")
                    if nh == 0:
                        nc.vector.tensor_copy(out=stg[:], in_=p[:])
                    else:
                        nc.scalar.copy(out=stg[:], in_=p[:])
                    nc.sync.dma_start(
                        out=out_d[c * 128:(c + 1) * 128, nh * 512:(nh + 1) * 512],
                        in_=stg[:],
                    )

            for c in range(NCH):
                emit_gate(c)
                emit_front(c)
                if c > 0:
                    emit_back(c - 1)
                if c > 1:
                    emit_tail(c - 2)
            emit_back(NCH - 1)
            emit_tail(NCH - 2)
            emit_tail(NCH - 1)

    nc.compile()
    return nc


_NC_CACHE = None


def _get_program():
    global _NC_CACHE
    if _NC_CACHE is None:
        _NC_CACHE = build_program()
    return _NC_CACHE


def shard_inputs(
    src, valid_mask, Wq, Wk, Wv, conv_q_w, conv_k_w, conv_v_w,
    Wg1, Wg2, bg2, Wgate, rms_w, Wo,
):
    bf = ml_dtypes.bfloat16
    f = np.float32

    def pack_km(w, m):
        return np.ascontiguousarray(
            np.asarray(w, f).reshape(8, 128, m).transpose(1, 0, 2).reshape(128, 8 * m)
        ).astype(bf)

    src = np.asarray(src, f)
    valid_mask = np.asarray(valid_mask)
    wo_scaled = np.asarray(Wo, f) * np.tile(np.asarray(rms_w, f), VD // DV)[:, None]
    in_maps = []
    for core in range(NCORES):
        b, hg = core // 2, core % 2
        qs = slice(hg * KDC, (hg + 1) * KDC)
        vs = slice(hg * VDC, (hg + 1) * VDC)
        wg2b = np.concatenate(
            [np.asarray(Wg2, f)[:, qs], np.asarray(bg2, f)[None, qs]], axis=0
        )
        convw = np.zeros((128, 32), f)
        ti = 0
        for w, sel, n in ((conv_q_w, qs, MIQ), (conv_k_w, qs, MIQ),
                          (conv_v_w, vs, MIV)):
            wa = np.asarray(w, f)[sel]
            for i in range(n):
                convw[:, ti * 4:(ti + 1) * 4] = wa[i * 128:(i + 1) * 128]
                ti += 1
        wo_core = np.ascontiguousarray(
            wo_scaled[vs].reshape(4, 128, 1024).transpose(1, 0, 2).reshape(128, 4096)
        ).astype(bf)
        in_maps.append({
            "srcT_in": np.ascontiguousarray(
                src[b].T.reshape(8, 128, T).transpose(1, 0, 2).reshape(128, 8 * T)
            ).astype(bf),
            "wq": pack_km(np.asarray(Wq, f)[:, qs], 256),
            "wk": pack_km(np.asarray(Wk, f)[:, qs], 256),
            "wv": pack_km(np.asarray(Wv, f)[:, vs], 512),
            "wgate": pack_km(np.asarray(Wgate, f)[:, vs], 512),
            "wg1": pack_km(np.asarray(Wg1, f), 16),
            "wg2b": np.ascontiguousarray(wg2b).astype(bf),
            "wo": wo_core,
            "convw": convw,
            "maskc": np.ascontiguousarray(
                valid_mask[b].astype(f).reshape(NCH, 128).T
            ),
        })
    return in_maps


def kernel(**inputs):
    nc = _get_program()
    in_maps = shard_inputs(**inputs)
    res = run_bass_kernel_spmd(nc, in_maps, list(range(NCORES)))
    out = np.zeros((B, T, D), np.float32)
    for core in range(NCORES):
        out[core // 2] += res.results[core]["out"]
    return out


if __name__ == "__main__":
    prog = _get_program()
    print("program built OK")


# revision 15
# speedup vs baseline: 1.3723x; 1.0092x over previous
"""Gated Linear Attention forward on 8 Trainium2 NeuronCores (Bass/Tile).

Problem: B=4, T=1024, D=1024, H=8, DK=64, DV=128, conv4 on q/k/v, low-rank
log-sigmoid forget gate, recurrent scan, RMS-norm + swish output gate, out proj.

Sharding: core = 2*b + hg  (b = batch, hg = half of the heads).
Each core computes its batch's tokens for 4 heads end-to-end and a partial
output projection (Wo row-block); the host sums the two partials per batch.

v4: bf16 matmuls (fp32 PSUM), and:
 - src arrives as 16 half-tiles issued first across all three DMA queues;
   weight DMAs are single straight descriptors (no AP splitting).
 - depthwise conv4 runs on DVE/GpSimd as shifted scalar_tensor_tensor chains
   in fp16 (frees ~20us of PE diag-matmul time).
 - silu is a single ACT op where available; the ACT table schedule is
   exp -> ln -> exp -> silu with NOTHING in the chunk loop but filler
   functions (square/copy), so zero in-loop table loads.
 - the per-chunk RMS rsqrt is computed on DVE (bit-trick seed + 2 Newton
   steps) instead of ACT Ln/Exp, which would force 4 table loads per chunk.
 - the output gate projection is interleaved chunk-by-chunk with the GLA
   recurrence, and each chunk's rms/gate/transpose/out-projection tail is
   emitted one chunk behind the recurrence so the Newton-rr latency stays
   off the critical path; output streams to DRAM throughout.
"""

import numpy as np
import ml_dtypes

import concourse.bass as bass
import concourse.mybir as mybir
import concourse.tile as tile
from concourse import bacc
from concourse.bass_utils import run_bass_kernel_spmd

F32 = mybir.dt.float32
BF = mybir.dt.bfloat16
F16 = mybir.dt.float16
I32 = mybir.dt.int32
AF = mybir.ActivationFunctionType
OP = mybir.AluOpType

B, T, D, H = 4, 1024, 1024, 8
KD, VD = 512, 1024
DK, DV = 64, 128
CONV = 4
GATE_NORM = 16.0
EPS = 1e-5
LN8 = float(np.log(8.0))

KDC, VDC = 256, 512
MIQ, MIV = 2, 4
C, NCH = 128, 8
G = 2
NCORES = 8

RSQRT_MAGIC = 0x5F3759DF
import os as _os
USE_SILU = _os.environ.get("K_NO_SILU") != "1"   # HW-verified; CoreSim lacks Silu


def build_program():
    nc = bacc.Bacc("TRN2", target_bir_lowering=False, debug=False)

    srcT_d = nc.dram_tensor("srcT_in", [128, 8 * T], BF, kind="ExternalInput")
    wq_d = nc.dram_tensor("wq", [128, 8 * 256], BF, kind="ExternalInput")
    wk_d = nc.dram_tensor("wk", [128, 8 * 256], BF, kind="ExternalInput")
    wv_d = nc.dram_tensor("wv", [128, 8 * 512], BF, kind="ExternalInput")
    wgate_d = nc.dram_tensor("wgate", [128, 8 * 512], BF, kind="ExternalInput")
    wg1_d = nc.dram_tensor("wg1", [128, 8 * 16], BF, kind="ExternalInput")
    wg2b_d = nc.dram_tensor("wg2b", [17, KDC], BF, kind="ExternalInput")
    wo_d = nc.dram_tensor("wo", [128, 4 * 1024], BF, kind="ExternalInput")
    convw_d = nc.dram_tensor("convw", [128, 32], F32, kind="ExternalInput")
    maskc_d = nc.dram_tensor("maskc", [128, NCH], F32, kind="ExternalInput")
    out_d = nc.dram_tensor("out", [T, D], F32, kind="ExternalOutput")

    ident_np = np.eye(128, dtype=ml_dtypes.bfloat16)
    u = np.triu(np.ones((128, 128), np.float32)).astype(ml_dtypes.bfloat16)
    ident_d = nc.inline_tensor(ident_np, "ident_c")
    triu2_d = nc.inline_tensor(np.concatenate([u, u], axis=1), "triu2_c")

    srcT = nc.alloc_sbuf_tensor("srcT", [128, 8, T], BF)
    q_sb = nc.alloc_sbuf_tensor("q_sb", [128, MIQ, T], BF)
    k_sb = nc.alloc_sbuf_tensor("k_sb", [128, MIQ, T], BF)
    v_sb = nc.alloc_sbuf_tensor("v_sb", [128, MIV, T], BF)
    gate_sb = nc.alloc_sbuf_tensor("gate_sb", [128, NCH, VDC], BF)
    xgT = nc.alloc_sbuf_tensor("xgT", [17, T], BF)
    spT = nc.alloc_sbuf_tensor("spT", [128, MIQ, T], F32)
    bsum = nc.alloc_sbuf_tensor("bsum", [128, MIQ, T], F32)
    texpq = nc.alloc_sbuf_tensor("texpq", [128, MIQ, T], BF)
    texpk = nc.alloc_sbuf_tensor("texpk", [128, MIQ, T], BF)
    Eall = nc.alloc_sbuf_tensor("Eall", [128, MIQ, NCH], F32)
    ssq_all = nc.alloc_sbuf_tensor("ssq_all", [128, NCH * 4], F32)
    rr_all = nc.alloc_sbuf_tensor("rr_all", [128, NCH * 4], F32)
    wq_sb = nc.alloc_sbuf_tensor("wq_sb", [128, 8, 256], BF)
    wk_sb = nc.alloc_sbuf_tensor("wk_sb", [128, 8, 256], BF)
    wv_sb = nc.alloc_sbuf_tensor("wv_sb", [128, 8, 512], BF)
    wgate_sb = nc.alloc_sbuf_tensor("wgate_sb", [128, 8, 512], BF)
    wg1_sb = nc.alloc_sbuf_tensor("wg1_sb", [128, 8, 16], BF)
    wg2b_sb = nc.alloc_sbuf_tensor("wg2b_sb", [17, KDC], BF)
    wo_sb = nc.alloc_sbuf_tensor("wo_sb", [128, 4, 1024], BF)
    convw_sb = nc.alloc_sbuf_tensor("convw_sb", [128, 32], F32)
    dg_sb = nc.alloc_sbuf_tensor("dg_sb", [128, 16, 128], BF)
    maskc_sb = nc.alloc_sbuf_tensor("maskc_sb", [128, NCH], F32)
    ident = nc.alloc_sbuf_tensor("ident", [128, 128], BF)
    triu2 = nc.alloc_sbuf_tensor("triu2", [128, 256], BF)
    ones_sb = nc.alloc_sbuf_tensor("ones_sb", [128, 128], F32)
    Sblk = [nc.alloc_sbuf_tensor(f"Sblk{g}", [128, 256], BF) for g in range(G)]
    qblk = [nc.alloc_sbuf_tensor(f"qblk{g}", [128, 256], BF) for g in range(G)]
    negln8 = nc.alloc_sbuf_tensor("negln8", [128, 1], F32)
    magic4 = nc.alloc_sbuf_tensor("magic4", [128, 4], I32)
    nhalf4 = nc.alloc_sbuf_tensor("nhalf4", [128, 4], F32)
    c15_4 = nc.alloc_sbuf_tensor("c15_4", [128, 4], F32)
    eps_col = nc.alloc_sbuf_tensor("eps_col", [128, 1], F32)

    with tile.TileContext(nc) as tc:
        with (
            tc.tile_pool(name="scr", bufs=4) as scr,
            tc.tile_pool(name="scr2", bufs=4) as scr2,
            tc.tile_pool(name="nwt", bufs=2) as nwt,
            tc.tile_pool(name="stage", bufs=4) as stage_pool,
            tc.tile_pool(name="ps_big", bufs=3, space="PSUM") as ps_big,
            tc.tile_pool(name="ps_sm", bufs=3, space="PSUM") as ps_sm,
            tc.tile_pool(name="ps_op", bufs=2, space="PSUM") as ps_op,
        ):
            # ---- DMA schedule: src halves first, round-robin over queues ---
            nc.sync.dma_start(out=wg1_sb[:].rearrange("p a b -> p (a b)"), in_=wg1_d[:])
            dma_engs = [nc.sync, nc.scalar, nc.gpsimd]
            for hf in range(16):
                kt, nh = hf // 2, hf % 2
                dma_engs[hf % 3].dma_start(
                    out=srcT[:, kt, nh * 512:(nh + 1) * 512],
                    in_=srcT_d[:, kt * T + nh * 512:kt * T + (nh + 1) * 512],
                )
            nc.sync.dma_start(out=wg2b_sb[:], in_=wg2b_d[:])
            nc.scalar.dma_start(out=convw_sb[:], in_=convw_d[:])
            nc.scalar.dma_start(out=maskc_sb[:], in_=maskc_d[:])
            nc.gpsimd.dma_start(out=ident[:], in_=ident_d[:])
            nc.gpsimd.dma_start(out=triu2[:], in_=triu2_d[:])
            nc.scalar.dma_start(out=wq_sb[:].rearrange("p a b -> p (a b)"), in_=wq_d[:])
            nc.sync.dma_start(out=wk_sb[:].rearrange("p a b -> p (a b)"), in_=wk_d[:])
            nc.gpsimd.dma_start(out=wv_sb[:].rearrange("p a b -> p (a b)"), in_=wv_d[:])
            nc.gpsimd.dma_start(out=wgate_sb[:].rearrange("p a b -> p (a b)"), in_=wgate_d[:])
            nc.gpsimd.dma_start(out=wo_sb[:].rearrange("p a b -> p (a b)"), in_=wo_d[:])

            nc.vector.memset(ones_sb[:], 1.0)
            nc.vector.memset(xgT[:], 1.0)
            nc.vector.memset(negln8[:], -LN8)
            nc.vector.memset(magic4[:], RSQRT_MAGIC)
            nc.vector.memset(nhalf4[:], -0.5)
            nc.vector.memset(c15_4[:], 1.5)
            nc.vector.memset(eps_col[:], EPS)
            for g in range(G):
                nc.vector.memset(Sblk[g][:], 0.0)
                nc.vector.memset(qblk[g][:], 0.0)

            # ---- gk path -----------------------------------------------------
            for nh in range(2):
                p = ps_big.tile([16, 512], F32, name="pp_xg", tag="ppb")
                for kt in range(8):
                    nc.tensor.matmul(
                        p[:], wg1_sb[:, kt, :], srcT[:, kt, nh * 512:(nh + 1) * 512],
                        start=(kt == 0), stop=(kt == 7),
                    )
                nc.scalar.copy(out=xgT[0:16, nh * 512:(nh + 1) * 512], in_=p[:])
            enxs = []
            for mi in range(MIQ):
                for nh in range(2):
                    p = ps_big.tile([128, 512], F32, name="pp_sp", tag="ppb")
                    nc.tensor.matmul(
                        p[:], wg2b_sb[:, mi * 128:(mi + 1) * 128],
                        xgT[:, nh * 512:(nh + 1) * 512], start=True, stop=True,
                    )
                    enx = scr2.tile([128, 512], BF, name="enx", tag="enx", bufs=4)
                    nc.scalar.activation(enx[:], p[:], AF.Exp, scale=-1.0)
                    enxs.append((mi, nh, enx))
            for mi, nh, enx in enxs:
                nc.scalar.activation(
                    spT[:, mi, nh * 512:(nh + 1) * 512], enx[:], AF.Ln, bias=1.0,
                )
            # ---- projections; conv4 on DVE/GpSimd in fp16 -------------------
            def conv_proj(w_sb, dst, mi_count, ti_base, conv_eng_for_mi,
                          evac_act=False, pe_conv=False):
                for mi in range(mi_count):
                    ceng = conv_eng_for_mi(mi)
                    pre = scr2.tile([128, 1028], BF if pe_conv else F16,
                                    name="pre", tag="pre")
                    nc.gpsimd.memset(pre[:, 0:4], 0.0)
                    for nh in range(2):
                        p = ps_big.tile([128, 512], F32, name="pp_c", tag="ppb")
                        for kt in range(8):
                            nc.tensor.matmul(
                                p[:], w_sb[:, kt, mi * 128:(mi + 1) * 128],
                                srcT[:, kt, nh * 512:(nh + 1) * 512],
                                start=(kt == 0), stop=(kt == 7),
                            )
                        if evac_act:
                            nc.scalar.copy(
                                out=pre[:, 4 + nh * 512:4 + (nh + 1) * 512], in_=p[:]
                            )
                        else:
                            nc.vector.tensor_copy(
                                out=pre[:, 4 + nh * 512:4 + (nh + 1) * 512], in_=p[:]
                            )
                    if pe_conv:
                        for nh in range(2):
                            seg = slice(nh * 512, (nh + 1) * 512)
                            cp = ps_big.tile([128, 512], F32, name="cp", tag="ppb")
                            for j in range(CONV):
                                nc.tensor.matmul(
                                    cp[:], dg_sb[:, mi * 4 + j, :],
                                    pre[:, 1 + nh * 512 + j:1 + nh * 512 + j + 512],
                                    start=(j == 0), stop=(j == 3),
                                )
                            nc.scalar.activation(dst[:, mi, seg], cp[:], AF.Silu)
                        continue
                    acc = scr2.tile([128, 1024], F16, name="acc", tag="acc")
                    for nh in range(2):
                        seg = slice(nh * 512, (nh + 1) * 512)
                        t4 = (ti_base + mi) * 4
                        nc_e = ceng
                        nc_e.tensor_scalar_mul(
                            acc[:, seg], pre[:, 1 + nh * 512:1 + nh * 512 + 512],
                            convw_sb[:, t4:t4 + 1],
                        )
                        for j in range(1, CONV):
                            nc_e.scalar_tensor_tensor(
                                out=acc[:, seg],
                                in0=pre[:, 1 + nh * 512 + j:1 + nh * 512 + j + 512],
                                scalar=convw_sb[:, t4 + j:t4 + j + 1],
                                in1=acc[:, seg], op0=OP.mult, op1=OP.add,
                            )
                        if USE_SILU:
                            nc.scalar.activation(
                                dst[:, mi, seg], acc[:, seg], AF.Silu,
                            )
                        else:
                            sg = scr2.tile([128, 512], BF, name="sg", tag="sg")
                            nc.scalar.activation(sg[:], acc[:, seg], AF.Sigmoid)
                            nc.vector.tensor_mul(dst[:, mi, seg], acc[:, seg], sg[:])

            conv_proj(wq_sb, q_sb, MIQ, 0, lambda mi: nc.vector)
            conv_proj(wk_sb, k_sb, MIQ, MIQ, lambda mi: nc.vector)
            for mi in range(MIQ):
                for c in range(NCH):
                    csl = slice(c * 128, (c + 1) * 128)
                    nc.vector.tensor_tensor_scan(
                        out=bsum[:, mi, csl], data0=ones_sb[:], data1=spT[:, mi, csl],
                        initial=0.0, op0=OP.mult, op1=OP.add,
                    )
                ends = bsum[:, mi, :].rearrange("p (c s) -> p c s", s=128)[:, :, 127:128]
                bce = scr.tile([128, 8], F32, name="bce", tag="bce")
                nc.vector.tensor_copy(
                    out=bce[:].rearrange("p (c one) -> p c one", one=1), in_=ends
                )
                nc.scalar.activation(Eall[:, mi, :], bce[:], AF.Exp, scale=-1.0 / GATE_NORM)
                nc.scalar.activation(
                    texpq[:, mi, :], bsum[:, mi, :], AF.Exp,
                    scale=-1.0 / GATE_NORM, bias=negln8[:],
                )
                nc.scalar.activation(
                    texpk[:, mi, :], bsum[:, mi, :], AF.Exp, scale=1.0 / GATE_NORM,
                )

            for mi in range(MIQ):
                nc.vector.tensor_mul(q_sb[:, mi, :], q_sb[:, mi, :], texpq[:, mi, :])
                nc.vector.tensor_mul(k_sb[:, mi, :], k_sb[:, mi, :], texpk[:, mi, :])
            for t4 in range(16):
                nc.vector.tensor_scalar_mul(
                    dg_sb[:, t4, :], ident[:], convw_sb[:, 16 + t4:16 + t4 + 1]
                )
            conv_proj(wv_sb, v_sb, MIV, 2 * MIQ, lambda mi: nc.vector,
                      evac_act=True, pe_conv=True)

            # ---- chunk loop: gate proj + recurrence + (lagged) fused tail ---
            def emit_gate(mt):
                p = ps_big.tile([128, 512], F32, name="pp_g", tag="ppb")
                for kt in range(8):
                    nc.tensor.matmul(
                        p[:], srcT[:, kt, mt * 128:(mt + 1) * 128], wgate_sb[:, kt, :],
                        start=(kt == 0), stop=(kt == 7),
                    )
                if USE_SILU:
                    nc.scalar.activation(gate_sb[:, mt, :], p[:], AF.Silu)
                else:
                    sgg = scr2.tile([128, 512], BF, name="sgg", tag="sg")
                    nc.scalar.activation(sgg[:], p[:], AF.Sigmoid)
                    nc.vector.tensor_mul(gate_sb[:, mt, :], p[:], sgg[:])

            fronts = {}

            def emit_front(c):
                csl = slice(c * 128, (c + 1) * 128)
                front = []
                for g in range(G):
                    e_col = Eall[:, g, c:c + 1]
                    kh_s = scr.tile([128, 128], BF, name="kh_s", tag="kh_s")
                    nc.vector.tensor_scalar_mul(kh_s[:], k_sb[:, g, csl], e_col)
                    nc.vector.tensor_copy(out=qblk[g][0:64, 0:128], in_=q_sb[0:64, g, csl])
                    nc.vector.tensor_copy(out=qblk[g][64:128, 128:256], in_=q_sb[64:128, g, csl])
                    ps_a = ps_sm.tile([128, 256], F32, name="ps_a", tag="ps_sm")
                    nc.tensor.matmul(
                        ps_a[:], k_sb[:, g, csl], qblk[g][:], start=True, stop=True,
                    )
                    a_sb = scr.tile([128, 256], BF, name="a_sb", tag="a_sb")
                    nc.vector.tensor_mul(a_sb[:], ps_a[:], triu2[:])
                    ps_v = ps_sm.tile([128, 256], BF, name="ps_v", tag="ps_sm")
                    nc.tensor.matmul(
                        ps_v[:, 0:128], v_sb[:, 2 * g, csl], ident[:],
                        is_transpose=True, start=True, stop=False, skip_group_check=True,
                    )
                    nc.tensor.matmul(
                        ps_v[:, 128:256], v_sb[:, 2 * g + 1, csl], ident[:],
                        is_transpose=True, start=False, stop=True, skip_group_check=True,
                    )
                    vnat = scr.tile([128, 256], BF, name="vnat", tag="vnat")
                    nc.vector.tensor_scalar_mul(vnat[:], ps_v[:], maskc_sb[:, c:c + 1])
                    ps_k = ps_sm.tile([128, 128], BF, name="ps_k", tag="ps_sm")
                    nc.tensor.transpose(ps_k[:], kh_s[:], ident[:])
                    khnat = scr.tile([128, 128], BF, name="khnat", tag="khnat")
                    nc.vector.tensor_copy(out=khnat[:], in_=ps_k[:])
                    front.append((a_sb, vnat, khnat, e_col))
                fronts[c] = front

            def emit_back(c):
                csl = slice(c * 128, (c + 1) * 128)
                front = fronts.pop(c)
                ps_os = []
                for g in range(G):
                    a_sb, vnat, khnat, e_col = front[g]
                    ps_o = ps_op.tile([128, 256], F32, name="ps_o", tag="ps_o")
                    nc.tensor.matmul(
                        ps_o[:, 0:128], a_sb[:, 0:128], vnat[:, 0:128],
                        start=True, stop=False, skip_group_check=True,
                    )
                    nc.tensor.matmul(
                        ps_o[:, 128:256], a_sb[:, 128:256], vnat[:, 128:256],
                        start=False, stop=False, skip_group_check=True,
                    )
                    nc.tensor.matmul(
                        ps_o[:], q_sb[:, g, csl], Sblk[g][:],
                        start=False, stop=True, skip_group_check=True,
                    )
                    ps_s = ps_sm.tile([128, 256], F32, name="ps_s", tag="ps_sm")
                    nc.tensor.matmul(ps_s[:], khnat[:], vnat[:], start=True, stop=True)
                    nc.vector.scalar_tensor_tensor(
                        out=Sblk[g][0:64, 0:128], in0=Sblk[g][0:64, 0:128],
                        scalar=e_col[0:64, :], in1=ps_s[0:64, 0:128],
                        op0=OP.mult, op1=OP.add,
                    )
                    nc.vector.scalar_tensor_tensor(
                        out=Sblk[g][64:128, 128:256], in0=Sblk[g][64:128, 128:256],
                        scalar=e_col[64:128, :], in1=ps_s[64:128, 128:256],
                        op0=OP.mult, op1=OP.add,
                    )
                    idx = c * 4 + 2 * g
                    for lh in range(2):
                        sqd = scr.tile([128, 128], BF, name="sqd", tag="sqd")
                        nc.scalar.activation(
                            sqd[:], ps_o[:, lh * 128:(lh + 1) * 128], AF.Square,
                            accum_out=ssq_all[:, idx + lh:idx + lh + 1],
                        )
                    ps_os.append(ps_o)
                # rr = rsqrt(ssq/DV + eps) on DVE: bit-trick seed + 2 Newton
                cs4 = slice(c * 4, c * 4 + 4)
                tloc = nwt.tile([128, 4], F32, name="tloc", tag="nt")
                nc.vector.tensor_scalar(
                    out=tloc[:], in0=ssq_all[:, cs4], scalar1=1.0 / DV, scalar2=EPS,
                    op0=OP.mult, op1=OP.add,
                )
                sh = nwt.tile([128, 4], I32, name="sh", tag="nsh")
                nc.vector.tensor_scalar(
                    out=sh[:], in0=tloc[:].bitcast(I32), scalar1=1, scalar2=None,
                    op0=OP.logical_shift_right,
                )
                yv = nwt.tile([128, 4], F32, name="yv", tag="ny")
                nc.gpsimd.tensor_tensor(
                    out=yv[:].bitcast(I32), in0=magic4[:], in1=sh[:], op=OP.subtract,
                )
                for _ in range(2):
                    y2 = nwt.tile([128, 4], F32, name="y2", tag="n2")
                    nc.gpsimd.tensor_mul(y2[:], yv[:], yv[:])
                    nc.gpsimd.tensor_mul(y2[:], y2[:], tloc[:])
                    nc.gpsimd.tensor_mul(y2[:], y2[:], nhalf4[:])
                    nc.gpsimd.tensor_tensor(
                        out=y2[:], in0=y2[:], in1=c15_4[:], op=OP.add,
                    )
                    yn = nwt.tile([128, 4], F32, name="yn", tag="ny")
                    nc.gpsimd.tensor_mul(yn[:], yv[:], y2[:])
                    yv = yn
                nc.gpsimd.tensor_copy(out=rr_all[:, cs4], in_=yv[:])
                # gated output: gate_sb <- (o * rr) * gate
                for g in range(G):
                    idx = c * 4 + 2 * g
                    for lh in range(2):
                        gdst = gate_sb[:, c, g * 256 + lh * 128:g * 256 + (lh + 1) * 128]
                        nc.vector.scalar_tensor_tensor(
                            out=gdst, in0=ps_os[g][:, lh * 128:(lh + 1) * 128],
                            scalar=rr_all[:, idx + lh:idx + lh + 1], in1=gdst,
                            op0=OP.mult, op1=OP.mult,
                        )

            def emit_tail(c):
                csl = slice(c * 128, (c + 1) * 128)
                ogs = []
                for hp in range(0, 4, 2):
                    ps_g = ps_sm.tile([128, 256], BF, name="ps_g", tag="ps_sm")
                    nc.tensor.matmul(
                        ps_g[:, 0:128], gate_sb[:, c, hp * 128:(hp + 1) * 128],
                        ident[:], is_transpose=True, start=True, stop=False,
                        skip_group_check=True,
                    )
                    nc.tensor.matmul(
                        ps_g[:, 128:256], gate_sb[:, c, (hp + 1) * 128:(hp + 2) * 128],
                        ident[:], is_transpose=True, start=False, stop=True,
                        skip_group_check=True,
                    )
                    og = scr.tile([128, 256], BF, name="og", tag="og")
                    nc.vector.tensor_copy(out=og[:], in_=ps_g[:])
                    ogs.append(og)
                for nh in range(2):
                    p = ps_big.tile([128, 512], F32, name="p_out", tag="ppb")
                    for h in range(4):
                        nc.tensor.matmul(
                            p[:], ogs[h // 2][:, (h % 2) * 128:(h % 2 + 1) * 128],
                            wo_sb[:, h, nh * 512:(nh + 1) * 512],
                            start=(h == 0), stop=(h == 3),
                        )
                    stg = stage_pool.tile([128, 512], F32, name="stage", tag="stage")
                    if nh == 0:
                        nc.vector.tensor_copy(out=stg[:], in_=p[:])
                    else:
                        nc.scalar.copy(out=stg[:], in_=p[:])
                    nc.sync.dma_start(
                        out=out_d[c * 128:(c + 1) * 128, nh * 512:(nh + 1) * 512],
                        in_=stg[:],
                    )

            for c in range(NCH):
                emit_gate(c)
                emit_front(c)
                if c > 0:
                    emit_back(c - 1)
                if c > 1:
                    emit_tail(c - 2)
            emit_back(NCH - 1)
            emit_tail(NCH - 2)
            emit_tail(NCH - 1)

    nc.compile()
    return nc


_NC_CACHE = None


def _get_program():
    global _NC_CACHE
    if _NC_CACHE is None:
        _NC_CACHE = build_program()
    return _NC_CACHE


def shard_inputs(
    src, valid_mask, Wq, Wk, Wv, conv_q_w, conv_k_w, conv_v_w,
    Wg1, Wg2, bg2, Wgate, rms_w, Wo,
):
    bf = ml_dtypes.bfloat16
    f = np.float32

    def pack_km(w, m):
        return np.ascontiguousarray(
            np.asarray(w, f).reshape(8, 128, m).transpose(1, 0, 2).reshape(128, 8 * m)
        ).astype(bf)

    src = np.asarray(src, f)
    valid_mask = np.asarray(valid_mask)
    wo_scaled = np.asarray(Wo, f) * np.tile(np.asarray(rms_w, f), VD // DV)[:, None]
    in_maps = []
    for core in range(NCORES):
        b, hg = core // 2, core % 2
        qs = slice(hg * KDC, (hg + 1) * KDC)
        vs = slice(hg * VDC, (hg + 1) * VDC)
        wg2b = np.concatenate(
            [np.asarray(Wg2, f)[:, qs], np.asarray(bg2, f)[None, qs]], axis=0
        )
        convw = np.zeros((128, 32), f)
        ti = 0
        for w, sel, n in ((conv_q_w, qs, MIQ), (conv_k_w, qs, MIQ),
                          (conv_v_w, vs, MIV)):
            wa = np.asarray(w, f)[sel]
            for i in range(n):
                convw[:, ti * 4:(ti + 1) * 4] = wa[i * 128:(i + 1) * 128]
                ti += 1
        wo_core = np.ascontiguousarray(
            wo_scaled[vs].reshape(4, 128, 1024).transpose(1, 0, 2).reshape(128, 4096)
        ).astype(bf)
        in_maps.append({
            "srcT_in": np.ascontiguousarray(
                src[b].T.reshape(8, 128, T).transpose(1, 0, 2).reshape(128, 8 * T)
            ).astype(bf),
            "wq": pack_km(np.asarray(Wq, f)[:, qs], 256),
            "wk": pack_km(np.asarray(Wk, f)[:, qs], 256),
            "wv": pack_km(np.asarray(Wv, f)[:, vs], 512),
            "wgate": pack_km(np.asarray(Wgate, f)[:, vs], 512),
            "wg1": pack_km(np.asarray(Wg1, f), 16),
            "wg2b": np.ascontiguousarray(wg2b).astype(bf),
            "wo": wo_core,
            "convw": convw,
            "maskc": np.ascontiguousarray(
                valid_mask[b].astype(f).reshape(NCH, 128).T
            ),
        })
    return in_maps


def kernel(**inputs):
    nc = _get_program()
    in_maps = shard_inputs(**inputs)
    res = run_bass_kernel_spmd(nc, in_maps, list(range(NCORES)))
    out = np.zeros((B, T, D), np.float32)
    for core in range(NCORES):
        out[core // 2] += res.results[core]["out"]
    return out


if __name__ == "__main__":
    prog = _get_program()
    print("program built OK")
